# revision 11
# baseline (speedup 1.0000x reference)
"""Distributed Trainium2 kernel for nn_DecoderAttentionRotary.

Strategy (8 NeuronCores, tensor-parallel over heads, fp16 matmul datapath):
  - host: transpose x -> xT [D, B*L] fp16; per-core Wqkv column slice
    reordered to [q0,k0,q1,k1,v0|v1] fp16; cos/sin transposed fp16; one
    128x128 causal mask; v-bias folded into bd (softmax rows sum to 1,
    so attn@(xWv+1*bv)@Wd = attn@xWv@Wd + bv@Wd).
  - device, per core (2 heads), with CROSS-PHASE INTERLEAVED emission
    (attention is ACT/exp-throughput-bound, projections are PE-bound):
      section A: qkv projection b0 (startup DMAs finely interleaved,
                 v-columns of W deferred),
      section B: attention(b0) interleaved ~1:2 with qkv projection(b1);
                 the b0 AllToAlls are fully hidden here,
      section C: attention(b1) compressed (fast A2As),
      section D: out-projection; b0's half hides the last A2A, and the
                 first 4 chains of b1's half are split even/odd k so the
                 even (head-0) halves also run before the last A2A lands.
  - attention: scores^T layout; k-tiles in PAIRS sharing one [128,1024]
    PSUM tile and ONE packed exp (amortizes the ~260ns ACT instruction
    overhead); out-matmuls lag 2 pairs behind scores; row-sums
    accumulated on DVE (sumacc += et), reduced by a single ones-matmul
    per q-chunk + fast approximate reciprocal + gpsimd broadcast, all
    staggered one q-chunk behind compute so nothing waits on the chain.
  - per-(batch,head) AllToAll reshard (fp16); o_sb reshard loads are
    FIFO-chained plain copies explicitly dep-ordered after the next
    head's attention-output DMAs (the tile scheduler otherwise hoists
    them where their semaphore waits head-of-line-block the sync queue);
    the final group is issued from the ACT hardware DMA queue.
  - y computed in fp16 (halves output DMA), upcast to fp32 on host.
  - host: scatter the per-core 256-row halves into the full output.
"""
import sys

for _p in ("/opt/pypackages", "/opt/trn_rl_repo"):
    if _p not in sys.path:
        sys.path.insert(0, _p)

import numpy as np

B, L, D, H = 2, 2048, 2048, 16
HD, R = 128, 32
SCALE = float(HD) ** -0.5
W = 8
HPC = H // W              # heads per core
M = B * L                 # flattened rows
CORES = list(range(W))

_NC = None


def _build_nc():
    import concourse.mybir as mybir
    import concourse.tile as tile
    from concourse import bacc
    from concourse.bass import _add_dep_helper
    from concourse.bass_isa import ReduceOp

    f32 = mybir.dt.float32
    f16 = mybir.dt.float16
    AFT = mybir.ActivationFunctionType
    OP = mybir.AluOpType

    nc = bacc.Bacc(None, target_bir_lowering=False, num_devices=W)
    xT = nc.declare_dram_parameter("xT", [D, M], f16, isOutput=False)
    wqkv = nc.declare_dram_parameter("wqkv", [D, 6 * HD], f16, isOutput=False)
    bqk = nc.declare_dram_parameter("bqk", [4 * HD, 1], f32, isOutput=False)
    cosT = nc.declare_dram_parameter("cosT", [R, L], f16, isOutput=False)
    sinT = nc.declare_dram_parameter("sinT", [R, L], f16, isOutput=False)
    mask0 = nc.declare_dram_parameter("mask0", [128, 128], f16, isOutput=False)
    wd = nc.declare_dram_parameter("wd", [D, D], f16, isOutput=False)
    bdb = nc.declare_dram_parameter("bdb", [128, D], f16, isOutput=False)
    y = nc.declare_dram_parameter("y", [M // W, D], f16, isOutput=True)

    xT_r = xT.ap().rearrange("(t p) n -> p t n", p=128)   # [128, 16, M]
    wq_r = wqkv.ap().rearrange("(t p) m -> p t m", p=128)
    wd_r = wd.ap().rearrange("(t p) n -> p t n", p=128)

    with tile.TileContext(nc) as tc:
        with (
            tc.tile_pool(name="const", bufs=1) as cpool,
            tc.tile_pool(name="dram", bufs=1, space="DRAM") as dpool,
            tc.tile_pool(name="ps", bufs=1, space="PSUM") as pp,
            tc.tile_pool(name="qkv", bufs=1) as qkvpool,
            tc.tile_pool(name="p3s", bufs=2) as p3s,
            tc.tile_pool(name="att", bufs=2) as apool,
            tc.tile_pool(name="p1", bufs=2) as p1pool,
        ):
            a2a_ins = [[dpool.tile([W, HD, 256], f16, name=f"a2ain{b}_{h}")
                        for h in range(HPC)] for b in range(B)]
            a2a_outs = [[dpool.tile([W, HD, 256], f16, name=f"a2aout{b}_{h}")
                         for h in range(HPC)] for b in range(B)]

            w_sb = cpool.tile([128, 16, 6 * HD], f16)
            wd_sb = cpool.tile([128, 16, D], f16)
            bd_sb = cpool.tile([128, D], f16)
            bqk_sb = cpool.tile([128, 4], f32)
            ones_c = cpool.tile([128, 1], f16)
            cos_sb = qkvpool.tile([R, L], f16, tag="shr", bufs=2,
                                  padded_shape=[128, 16 * 256])
            sin_sb = qkvpool.tile([R, L], f16, tag="shr", bufs=2,
                                  padded_shape=[128, 16 * 256])
            mask_sb = cpool.tile([128, 128], f16)
            qk_sbs, v_sbs = [], []
            for b in range(B):
                qk_sbs.append(qkvpool.tile([128, 4, L], f16, name=f"qk{b}"))
                v_sbs.append(qkvpool.tile([128, 16, 2 * HD], f16, name=f"v{b}"))

            # ---- startup ----
            # sync queue carries ONLY xt pieces (big, latency-critical);
            # weights/consts go on the scalar queue so neither FIFO
            # head-blocks the other.  xt double-buffer depth is 2 full
            # chunks (bufs=4) so a chunk's loads never WAR-wait on the
            # previous chunk's reads and always land early.
            xt_store = {}

            def load_xt(ci):
                b, nch = divmod(ci, 4)
                n0 = b * L + nch * 512
                tiles = []
                for half in range(2):
                    xt = p1pool.tile([128, 8, 512], f16, tag="xt", bufs=4,
                                     name=f"xt{ci}_{half}")
                    for piece in range(2):
                        t0 = half * 8 + piece * 4
                        nc.sync.dma_start(
                            out=xt[:, piece * 4:(piece + 1) * 4, :],
                            in_=xT_r[:, t0:t0 + 4, n0:n0 + 512],
                        )
                    tiles.append(xt)
                xt_store[ci] = tiles

            xt0_tiles = []
            for half in range(2):
                xt = p1pool.tile([128, 8, 512], f16, tag="xt", bufs=4,
                                 name=f"xt0_{half}")
                xt0_tiles.append(xt)
            # startup pieces interleaved on sync, consumption-ordered;
            # first pieces tiny so the first matmul can start ASAP
            def xt0_piece(t0, t1):
                hl = t0 // 8
                nc.sync.dma_start(
                    out=xt0_tiles[hl][:, t0 - 8 * hl:t1 - 8 * hl, :],
                    in_=xT_r[:, t0:t1, 0:512],
                )

            def w_piece(k0, k1):
                nc.sync.dma_start(
                    out=w_sb[:, k0:k1, 0:4 * HD],
                    in_=wq_r[:, k0:k1, 0:4 * HD],
                )

            xt0_piece(0, 1)
            w_piece(0, 1)
            w_piece(1, 4)
            xt0_piece(1, 4)
            w_piece(4, 8)
            xt0_piece(4, 8)
            w_piece(8, 12)
            xt0_piece(8, 12)
            w_piece(12, 16)
            xt0_piece(12, 16)
            xt_store[0] = xt0_tiles
            for half in range(2):
                nc.sync.dma_start(
                    out=w_sb[:, half * 8:(half + 1) * 8, 4 * HD:6 * HD],
                    in_=wq_r[:, half * 8:(half + 1) * 8, 4 * HD:6 * HD],
                )
            nc.sync.dma_start(
                out=bqk_sb[:], in_=bqk.ap().rearrange("(t p) o -> p (t o)", p=128)
            )
            load_xt(1)
            nc.vector.memset(ones_c[:], 1.0)
            # warmup matmuls: run during the startup DMA wait so the PE
            # HAM clock-gate is already at 8/8 when real work arrives.
            # bd_sb doubles as scratch: memset -> warm reads -> chunk-0 DMA
            # overwrites it (only consumed in section D).
            nc.vector.memset(bd_sb[:, 0:512], 1.0)
            wps = pp.tile([128, 512], f32, tag="work", bufs=2, name="warmps")
            for wi in range(10):
                nc.tensor.matmul(
                    wps[:], lhsT=bd_sb[:, 0:128], rhs=bd_sb[:, 0:512],
                    start=(wi == 0), stop=(wi == 9),
                )

            # ---- phase-1 chunk emission (generator; ~4-MM steps) ----
            def p1_chunk_steps(ci):
                b, nch = divmod(ci, 4)
                qk_sb, v_sb = qk_sbs[b], v_sbs[b]
                ch = slice(nch * 512, (nch + 1) * 512)
                if ci + 2 < 2 * (L // 512):
                    load_xt(ci + 2)
                if ci == 0:
                    nc.scalar.dma_start(out=cos_sb[:], in_=cosT.ap())
                    nc.scalar.dma_start(out=sin_sb[:], in_=sinT.ap())
                    nc.scalar.dma_start(out=mask_sb[:], in_=mask0.ap())
                    nc.scalar.dma_start(out=bd_sb[:], in_=bdb.ap())
                xt_tiles = xt_store.pop(ci)
                for mp in range(2):
                    ps2 = pp.tile([128, 1024], f32, tag="work", bufs=2,
                                  name=f"qkps{ci}_{mp}")
                    for kt in range(16):
                        xt = xt_tiles[kt // 8]
                        for i in range(2):
                            m = 2 * mp + i
                            nc.tensor.matmul(
                                ps2[:, i * 512:(i + 1) * 512],
                                lhsT=w_sb[:, kt, m * 128:(m + 1) * 128],
                                rhs=xt[:, kt % 8, :],
                                start=(kt == 0),
                                stop=(kt == 15),
                            )
                        if kt % 2 == 1:
                            yield
                    for i in range(2):
                        m = 2 * mp + i
                        nc.scalar.activation(
                            qk_sb[:, m, ch], ps2[:, i * 512:(i + 1) * 512],
                            AFT.Identity, bias=bqk_sb[:, m:m + 1],
                        )
                    yield
                for m in range(4):
                    # fused RoPE on rows 0:R (shuffle via DMA + 3 DVE ops)
                    cs = cos_sb[:, ch]
                    sn = sin_sb[:, ch]
                    rot = p1pool.tile([R, 512], f16, tag="rot", bufs=1)
                    # scalar DGE queue in section A (idle there); sync in B
                    # (scalar then carries the attention exps)
                    rq = nc.scalar if ci < 4 else nc.sync
                    rq.dma_start(out=rot[0:16, :], in_=qk_sb[16:32, m, ch])
                    rq.dma_start(out=rot[16:32, :], in_=qk_sb[0:16, m, ch])
                    nc.vector.tensor_tensor(
                        qk_sb[0:R, m, ch], qk_sb[0:R, m, ch], cs, op=OP.mult
                    )
                    nc.vector.tensor_tensor(rot[:], rot[:], sn, op=OP.mult)
                    nc.vector.tensor_tensor(
                        qk_sb[0:R, m, ch], qk_sb[0:R, m, ch], rot[:], op=OP.add
                    )
                    if m % 2 == 1:
                        yield
                for rr2 in range(2):
                    vpss = [
                        pp.tile([128, 2 * HD], f32,
                                tag=("work" if i else "acc"),
                                bufs=(2 if i else 3),
                                name=f"vps{ci}_{2 * rr2 + i}")
                        for i in range(2)
                    ]
                    for kt in range(16):
                        xt = xt_tiles[kt // 8]
                        for i in range(2):
                            rr = 2 * rr2 + i
                            nc.tensor.matmul(
                                vpss[i][:],
                                lhsT=xt[:, kt % 8, rr * 128:(rr + 1) * 128],
                                rhs=w_sb[:, kt, 4 * HD:6 * HD],
                                start=(kt == 0),
                                stop=(kt == 15),
                            )
                        if kt % 2 == 1:
                            yield
                    for i in range(2):
                        rr = 2 * rr2 + i
                        nc.scalar.activation(
                            v_sb[:, nch * 4 + rr, :], vpss[i][:], AFT.Copy
                        )
                    yield
                # stream Wd behind the x tiles
                nc.sync.dma_start(
                    out=wd_sb[:, 2 * ci:2 * (ci + 1), :],
                    in_=wd_r[:, 2 * ci:2 * (ci + 1), :],
                )

            # ---- attention emission (generator; yields per pair) ----
            last_ot = {}

            def attn_steps(b):
                qk_sb, v_sb = qk_sbs[b], v_sbs[b]
                for h in range(HPC):

                    def norm_a(state, b=b, h=h):
                        qc_p, outp_p, sumacc_p = state[:3]
                        sump = pp.tile([1, 512], f32, tag="sump", bufs=1,
                                       name=f"sump{b}_{h}_{qc_p}")
                        nc.tensor.matmul(
                            sump[:], lhsT=ones_c[:], rhs=sumacc_p[:],
                            start=True, stop=True,
                        )
                        rec1 = apool.tile([1, 512], f32, tag="bcs", bufs=1)
                        nc.vector.reciprocal_approx_fast(rec1[:], sump[:])
                        bcs = apool.tile([128, 512], f32, tag="rcp", bufs=1)
                        nc.gpsimd.partition_broadcast(bcs[:], rec1[:])
                        state.append(bcs)

                    def norm_b(state, b=b, h=h):
                        qc_p, outp_p, _, bcs = state
                        ot = apool.tile([128, 512], f16, tag="ot", bufs=2)
                        nc.vector.tensor_tensor(
                            ot[:], outp_p[:], bcs[:], op=OP.mult
                        )
                        for half in range(2):
                            last_ot[(b, h)] = nc.sync.dma_start(
                                out=a2a_ins[b][h][2 * qc_p + half, :, :],
                                in_=ot[:, half * 256:(half + 1) * 256],
                            )

                    prev = None   # [qc, outp, sumacc, (bcs)]
                    for qc in reversed(range(L // 512)):
                        nk = 4 * qc + 4
                        npairs = nk // 2
                        outp = pp.tile([128, 512], f32, tag="acc", bufs=3,
                                       name=f"outp{b}_{h}_{qc}")
                        sumacc = apool.tile([128, 512], f16, tag="sumacc",
                                            bufs=2)

                        def emit_out(pr, outp=outp, nk=nk, v_sb=v_sb, h=h):
                            ka, c0a, npra, kb, c0b, nprb, off, et2 = pr
                            nc.tensor.matmul(
                                outp[:, c0a:512],
                                lhsT=v_sb[:, ka, h * 128:(h + 1) * 128],
                                rhs=et2[:, 0:npra],
                                start=(ka == 0), stop=False,
                            )
                            nc.tensor.matmul(
                                outp[:, c0b:512],
                                lhsT=v_sb[:, kb, h * 128:(h + 1) * 128],
                                rhs=et2[:, off:off + nprb],
                                start=False, stop=(kb == nk - 1),
                            )

                        pend = []
                        for p in range(npairs):
                            ka, kb = 2 * p, 2 * p + 1
                            ja = max(0, ka - qc * 4)
                            jb = max(0, kb - qc * 4)
                            c0a, c0b = ja * 128, jb * 128
                            npra, nprb = 512 - c0a, 512 - c0b
                            off = npra
                            tot = npra + nprb
                            sp2 = pp.tile([128, 1024], f32, tag="work",
                                          bufs=2, name=f"sp{b}_{h}_{qc}_{p}")
                            nc.tensor.matmul(
                                sp2[:, 0:npra],
                                lhsT=qk_sb[:, 2 * h + 1,
                                           ka * 128:(ka + 1) * 128],
                                rhs=qk_sb[:, 2 * h,
                                          qc * 512 + c0a:(qc + 1) * 512],
                                start=True, stop=True,
                            )
                            nc.tensor.matmul(
                                sp2[:, off:off + nprb],
                                lhsT=qk_sb[:, 2 * h + 1,
                                           kb * 128:(kb + 1) * 128],
                                rhs=qk_sb[:, 2 * h,
                                          qc * 512 + c0b:(qc + 1) * 512],
                                start=True, stop=True,
                            )
                            et2 = apool.tile([128, 1024], f16, tag="et",
                                             bufs=4)
                            nc.scalar.activation(
                                et2[:, 0:tot], sp2[:, 0:tot], AFT.Exp,
                                scale=SCALE,
                            )
                            if ka >= qc * 4:
                                nc.vector.tensor_tensor(
                                    et2[:, 0:128], et2[:, 0:128], mask_sb[:],
                                    op=OP.mult,
                                )
                            if kb >= qc * 4:
                                nc.vector.tensor_tensor(
                                    et2[:, off:off + 128],
                                    et2[:, off:off + 128], mask_sb[:],
                                    op=OP.mult,
                                )
                            if p == 0:
                                # first tile always spans all 512 q-cols
                                nc.vector.tensor_copy(
                                    sumacc[:], et2[:, 0:512]
                                )
                            else:
                                nc.vector.tensor_tensor(
                                    sumacc[:, c0a:512], sumacc[:, c0a:512],
                                    et2[:, 0:npra], op=OP.add,
                                )
                            nc.vector.tensor_tensor(
                                sumacc[:, c0b:512], sumacc[:, c0b:512],
                                et2[:, off:off + nprb], op=OP.add,
                            )
                            pend.append(
                                (ka, c0a, npra, kb, c0b, nprb, off, et2)
                            )
                            if len(pend) > 2:
                                emit_out(pend.pop(0))
                            if p == 1 and prev is not None:
                                norm_a(prev)
                            if p == 3 and prev is not None:
                                norm_b(prev)
                                prev = None
                            yield
                        for pr in pend:
                            emit_out(pr)
                            yield
                        if prev is not None:   # qc with only 2 pairs
                            norm_b(prev)
                        prev = [qc, outp, sumacc]
                    norm_a(prev)
                    norm_b(prev)
                    prev = None
                    nc.gpsimd.collective_compute(
                        "AllToAll",
                        mybir.AluOpType.bypass,
                        replica_groups=[CORES],
                        ins=[a2a_ins[b][h][:]],
                        outs=[a2a_outs[b][h][:]],
                    )
                    yield

            # ---- o_sb reshard load groups ----
            o_sbs = [None, None]
            osb_chain = [None]

            def emit_osb_group(b, h, gate, engine):
                if o_sbs[b] is None:
                    o_sbs[b] = qkvpool.tile([128, 16, 256], f16, tag="shr",
                                            bufs=2, name=f"osb{b}")
                for jsrc in range(W):
                    ld = engine.dma_start(
                        out=o_sbs[b][:, 2 * jsrc + h, :],
                        in_=a2a_outs[b][h][jsrc, :, :],
                    )
                    if gate is not None:
                        _add_dep_helper(
                            ld.ins, gate.ins, sync=True,
                            reason="order o_sb after time-critical DMAs",
                        )
                    if engine is nc.sync:
                        if osb_chain[0] is not None:
                            _add_dep_helper(
                                ld.ins, osb_chain[0].ins, sync=True,
                                reason="FIFO-chain o_sb loads",
                            )
                        osb_chain[0] = ld

            # ---- phase-3 emission (generator; yields per half-chain) ----
            def p3_steps(bh):
                for n4 in range(4):
                    for i in range(2):
                        m = 2 * bh + i
                        yp = pp.tile([128, 512], f32, tag="work", bufs=2,
                                     name=f"yps{bh}_{n4}_{m}")
                        for kt in range(16):
                            nc.tensor.matmul(
                                yp[:],
                                lhsT=o_sbs[bh][:, kt, i * 128:(i + 1) * 128],
                                rhs=wd_sb[:, kt, n4 * 512:(n4 + 1) * 512],
                                start=(kt == 0), stop=(kt == 15),
                            )
                            if kt == 7:
                                yield
                        yt = p3s.tile([128, 512], f16, tag="yt", bufs=2)
                        nc.vector.tensor_tensor(
                            yt[:], yp[:], bd_sb[:, n4 * 512:(n4 + 1) * 512],
                            op=OP.add,
                        )
                        nc.sync.dma_start(
                            out=y[m * 128:(m + 1) * 128,
                                  n4 * 512:(n4 + 1) * 512],
                            in_=yt[:],
                        )
                        yield

            def drain(gen):
                for _ in gen:
                    pass

            def pull(gen, n):
                for _ in range(n):
                    if next(gen, StopIteration) is StopIteration:
                        return False
                return True

            # ======== section A: qkv projection b0 ========
            for ci in range(4):
                drain(p1_chunk_steps(ci))

            # ======== section B: attention(b0) x qkv projection(b1) ======
            def p1_rest():
                for ci in range(4, 8):
                    for _ in p1_chunk_steps(ci):
                        yield

            gA = attn_steps(0)
            gP = p1_rest()
            alive = True
            for _ in gA:
                if alive:
                    alive = pull(gP, 2)
            if alive:
                drain(gP)

            # o_sb loads for b0h0 (sync queue, gated after b0h1's ot writes)
            emit_osb_group(0, 0, last_ot[(0, 1)], nc.sync)

            # ======== section C: attention(b1) x out-projection(b0) ======
            # p3(b0) chains are interleaved 1:2 into the attention stretch
            # once o_sb[0] is ready, so the PE never starves on the
            # exp/DVE-paced attention chain (a >3.4us PE gap would also
            # drop the HAM clock to 1.2GHz).
            gA = attn_steps(1)
            pull(gA, 14)
            emit_osb_group(0, 1, last_ot.get((1, 0)), nc.sync)
            pull(gA, 12)
            gP3 = p3_steps(0)
            alive3 = True
            cnt = 0
            for _ in gA:
                cnt += 1
                if alive3 and cnt % 2 == 0:
                    alive3 = pull(gP3, 1)

            # o_sb loads for b1: h0 on sync (gated), h1 on the ACT hw queue
            emit_osb_group(1, 0, last_ot[(1, 1)], nc.sync)
            emit_osb_group(1, 1, None, nc.scalar)

            # ======== section D: out-projection (b0 hides the last A2A) ==
            if alive3:
                drain(gP3)
            # phase3-b1 wave A: 4 chains split even/odd kt. Even kt = head-0
            # channels (ready after the 3rd A2A), so the PE keeps working
            # while the last A2A is still in flight.
            waveA = [(0, 0), (0, 1), (1, 0), (1, 1)]
            ypw = {}
            for idx, (n4, i) in enumerate(waveA):
                yp = pp.tile([128, 512], f32,
                             tag=("work" if idx < 2 else "acc"),
                             bufs=(2 if idx < 2 else 3),
                             name=f"ypw{n4}_{i}")
                for kt in range(0, 16, 2):
                    nc.tensor.matmul(
                        yp[:],
                        lhsT=o_sbs[1][:, kt, i * 128:(i + 1) * 128],
                        rhs=wd_sb[:, kt, n4 * 512:(n4 + 1) * 512],
                        start=(kt == 0), stop=False,
                    )
                ypw[(n4, i)] = yp
            for (n4, i) in waveA:
                yp = ypw[(n4, i)]
                m = 2 + i
                for kt in range(1, 16, 2):
                    nc.tensor.matmul(
                        yp[:],
                        lhsT=o_sbs[1][:, kt, i * 128:(i + 1) * 128],
                        rhs=wd_sb[:, kt, n4 * 512:(n4 + 1) * 512],
                        start=False, stop=(kt == 15),
                    )
                yt = p3s.tile([128, 512], f16, tag="yt", bufs=2)
                nc.vector.tensor_tensor(
                    yt[:], yp[:], bd_sb[:, n4 * 512:(n4 + 1) * 512],
                    op=OP.add,
                )
                nc.sync.dma_start(
                    out=y[m * 128:(m + 1) * 128, n4 * 512:(n4 + 1) * 512],
                    in_=yt[:],
                )
            # wave B: remaining 4 chains, contiguous
            for n4 in (2, 3):
                for i in range(2):
                    m = 2 + i
                    yp = pp.tile([128, 512], f32, tag="work", bufs=2,
                                 name=f"ypb{n4}_{i}")
                    for kt in range(16):
                        nc.tensor.matmul(
                            yp[:],
                            lhsT=o_sbs[1][:, kt, i * 128:(i + 1) * 128],
                            rhs=wd_sb[:, kt, n4 * 512:(n4 + 1) * 512],
                            start=(kt == 0), stop=(kt == 15),
                        )
                    yt = p3s.tile([128, 512], f16, tag="yt", bufs=2)
                    nc.vector.tensor_tensor(
                        yt[:], yp[:], bd_sb[:, n4 * 512:(n4 + 1) * 512],
                        op=OP.add,
                    )
                    nc.sync.dma_start(
                        out=y[m * 128:(m + 1) * 128,
                              n4 * 512:(n4 + 1) * 512],
                        in_=yt[:],
                    )
    nc.finalize()
    return nc


def _host_prep(x_BLD, cos, sin, Wqkv, bqkv, Wd, bd):
    x = np.asarray(x_BLD, np.float32).reshape(M, D)
    xT = np.ascontiguousarray(x.T.astype(np.float16))
    cosT = np.ascontiguousarray(
        np.asarray(cos, np.float32).reshape(L, R).T.astype(np.float16)
    )
    s2 = np.asarray(sin, np.float32).reshape(L, R).T
    sinT_pm = np.ascontiguousarray(
        np.concatenate([-s2[:16], s2[16:]], axis=0).astype(np.float16)
    )
    kk = np.arange(128, dtype=np.int64)[:, None]
    qq = np.arange(128, dtype=np.int64)[None, :]
    mask0 = np.ascontiguousarray((qq >= kk).astype(np.float16))
    Wqkv = np.asarray(Wqkv, np.float32)
    bqkv = np.asarray(bqkv, np.float32)
    # softmax rows sum to 1, so the v-bias contribution collapses to a
    # constant vector bv_glob @ Wd folded into bd
    HDl = D // H
    v_bias = np.concatenate(
        [bqkv[hh * 3 * HDl + 2 * HDl:(hh * 3 + 3) * HDl] for hh in range(H)]
    )
    bd_eff = np.asarray(bd, np.float32) + v_bias @ np.asarray(Wd, np.float32)
    bdb = np.ascontiguousarray(
        np.broadcast_to(bd_eff.astype(np.float16), (128, D))
    )
    in_maps = []
    for c in range(W):
        base = c * HPC * 3 * HD
        qk_idx = np.concatenate(
            [np.arange(base + h * 3 * HD, base + h * 3 * HD + 2 * HD)
             for h in range(HPC)]
        )
        v_idx = np.concatenate(
            [np.arange(base + h * 3 * HD + 2 * HD, base + (h + 1) * 3 * HD)
             for h in range(HPC)]
        )
        in_maps.append({
            "xT": xT,
            "wqkv": np.ascontiguousarray(
                Wqkv[:, np.concatenate([qk_idx, v_idx])].astype(np.float16)
            ),
            "bqk": np.ascontiguousarray(bqkv[qk_idx].reshape(4 * HD, 1)),
            "cosT": cosT,
            "sinT": sinT_pm,
            "mask0": mask0,
            "wd": np.asarray(Wd, np.float32).astype(np.float16),
            "bdb": bdb,
        })
    return in_maps


def _get_nc():
    global _NC
    if _NC is None:
        _NC = _build_nc()
    return _NC


def _run(inputs, trace=False, tmpdir=None):
    from concourse.bass_utils import run_bass_kernel_spmd

    in_maps = _host_prep(**inputs)
    nc = _get_nc()
    res = run_bass_kernel_spmd(nc, in_maps, CORES, trace=trace, tmpdir=tmpdir)
    out = np.empty((M, D), np.float32)
    for c in CORES:
        yc = np.asarray(res.results[c]["y"], np.float32)  # [512, D]
        out[c * 256:(c + 1) * 256] = yc[:256]
        out[L + c * 256:L + (c + 1) * 256] = yc[256:]
    return out.reshape(B, L, D), res


def kernel(**inputs) -> np.ndarray:
    out, _ = _run(inputs)
    return out



# revision 12
# speedup vs baseline: 1.0581x; 1.0581x over previous
"""Distributed Trainium2 kernel for nn_DecoderAttentionRotary.

Strategy (8 NeuronCores, tensor-parallel over heads, fp16 matmul datapath):
  - host: transpose x -> xT [D, B*L] fp16; per-core Wqkv column slice
    reordered to [q0,k0,q1,k1,v0|v1] fp16; cos/sin transposed fp16; one
    128x128 causal mask; v-bias folded into bd (softmax rows sum to 1,
    so attn@(xWv+1*bv)@Wd = attn@xWv@Wd + bv@Wd).
  - device, per core (2 heads), with CROSS-PHASE INTERLEAVED emission
    (attention is ACT/exp-throughput-bound, projections are PE-bound):
      section A: qkv projection b0 (startup DMAs finely interleaved,
                 v-columns of W deferred),
      section B: attention(b0) interleaved ~1:2 with qkv projection(b1);
                 the b0 AllToAlls are fully hidden here,
      section C: attention(b1) compressed (fast A2As),
      section D: out-projection; b0's half hides the last A2A, and the
                 first 4 chains of b1's half are split even/odd k so the
                 even (head-0) halves also run before the last A2A lands.
  - attention: scores^T layout; k-tiles in PAIRS sharing one [128,1024]
    PSUM tile and ONE packed exp (amortizes the ~260ns ACT instruction
    overhead); out-matmuls lag 2 pairs behind scores; row-sums
    accumulated on DVE (sumacc += et), reduced by a single ones-matmul
    per q-chunk + fast approximate reciprocal + gpsimd broadcast, all
    staggered one q-chunk behind compute so nothing waits on the chain.
  - per-(batch,head) AllToAll reshard (fp16); o_sb reshard loads are
    FIFO-chained plain copies explicitly dep-ordered after the next
    head's attention-output DMAs (the tile scheduler otherwise hoists
    them where their semaphore waits head-of-line-block the sync queue);
    the final group is issued from the ACT hardware DMA queue.
  - y computed in fp16 (halves output DMA), upcast to fp32 on host.
  - host: scatter the per-core 256-row halves into the full output.
"""
import sys

for _p in ("/opt/pypackages", "/opt/trn_rl_repo"):
    if _p not in sys.path:
        sys.path.insert(0, _p)

import numpy as np

B, L, D, H = 2, 2048, 2048, 16
HD, R = 128, 32
SCALE = float(HD) ** -0.5
W = 8
HPC = H // W              # heads per core
M = B * L                 # flattened rows
CORES = list(range(W))

_NC = None


def _build_nc():
    import concourse.mybir as mybir
    import concourse.tile as tile
    from concourse import bacc
    from concourse.bass import _add_dep_helper
    from concourse.bass_isa import ReduceOp

    f32 = mybir.dt.float32
    f16 = mybir.dt.float16
    AFT = mybir.ActivationFunctionType
    OP = mybir.AluOpType

    nc = bacc.Bacc(None, target_bir_lowering=False, num_devices=W)
    xT = nc.declare_dram_parameter("xT", [D, M], f16, isOutput=False)
    wqkv = nc.declare_dram_parameter("wqkv", [D, 6 * HD], f16, isOutput=False)
    bqk = nc.declare_dram_parameter("bqk", [4 * HD, 1], f32, isOutput=False)
    cosT = nc.declare_dram_parameter("cosT", [R, L], f16, isOutput=False)
    sinT = nc.declare_dram_parameter("sinT", [R, L], f16, isOutput=False)
    mask0 = nc.declare_dram_parameter("mask0", [128, 128], f16, isOutput=False)
    wd = nc.declare_dram_parameter("wd", [D, D], f16, isOutput=False)
    bdb = nc.declare_dram_parameter("bdb", [128, D], f16, isOutput=False)
    y = nc.declare_dram_parameter("y", [M // W, D], f16, isOutput=True)

    xT_r = xT.ap().rearrange("(t p) n -> p t n", p=128)   # [128, 16, M]
    wq_r = wqkv.ap().rearrange("(t p) m -> p t m", p=128)
    wd_r = wd.ap().rearrange("(t p) n -> p t n", p=128)

    with tile.TileContext(nc) as tc:
        with (
            tc.tile_pool(name="const", bufs=1) as cpool,
            tc.tile_pool(name="dram", bufs=1, space="DRAM") as dpool,
            tc.tile_pool(name="ps", bufs=1, space="PSUM") as pp,
            tc.tile_pool(name="qkv", bufs=1) as qkvpool,
            tc.tile_pool(name="p3s", bufs=2) as p3s,
            tc.tile_pool(name="att", bufs=2) as apool,
            tc.tile_pool(name="p1", bufs=2) as p1pool,
        ):
            a2a_ins = [[dpool.tile([W, HD, 256], f16, name=f"a2ain{b}_{h}")
                        for h in range(HPC)] for b in range(B)]
            a2a_outs = [[dpool.tile([W, HD, 256], f16, name=f"a2aout{b}_{h}")
                         for h in range(HPC)] for b in range(B)]

            w_sb = cpool.tile([128, 16, 6 * HD], f16)
            wd_sb = cpool.tile([128, 16, D], f16)
            bd_sb = cpool.tile([128, D], f16)
            bqk_sb = cpool.tile([128, 4], f32)
            ones_c = cpool.tile([128, 1], f16)
            cos_sb = qkvpool.tile([R, L], f16, tag="shr", bufs=2,
                                  padded_shape=[128, 16 * 256])
            sin_sb = qkvpool.tile([R, L], f16, tag="shr", bufs=2,
                                  padded_shape=[128, 16 * 256])
            mask_sb = cpool.tile([128, 128], f16)
            qk_sbs, v_sbs = [], []
            for b in range(B):
                qk_sbs.append(qkvpool.tile([128, 4, L], f16, name=f"qk{b}"))
                v_sbs.append(qkvpool.tile([128, 16, 2 * HD], f16, name=f"v{b}"))

            # ---- startup ----
            # sync queue carries ONLY xt pieces (big, latency-critical);
            # weights/consts go on the scalar queue so neither FIFO
            # head-blocks the other.  xt double-buffer depth is 2 full
            # chunks (bufs=4) so a chunk's loads never WAR-wait on the
            # previous chunk's reads and always land early.
            xt_store = {}

            def load_xt(ci):
                b, nch = divmod(ci, 4)
                n0 = b * L + nch * 512
                tiles = []
                for half in range(2):
                    xt = p1pool.tile([128, 8, 512], f16, tag="xt", bufs=4,
                                     name=f"xt{ci}_{half}")
                    for piece in range(2):
                        t0 = half * 8 + piece * 4
                        nc.sync.dma_start(
                            out=xt[:, piece * 4:(piece + 1) * 4, :],
                            in_=xT_r[:, t0:t0 + 4, n0:n0 + 512],
                        )
                    tiles.append(xt)
                xt_store[ci] = tiles

            xt0_tiles = []
            for half in range(2):
                xt = p1pool.tile([128, 8, 512], f16, tag="xt", bufs=4,
                                 name=f"xt0_{half}")
                xt0_tiles.append(xt)
            # startup pieces interleaved on sync, consumption-ordered;
            # first pieces tiny so the first matmul can start ASAP
            def xt0_piece(t0, t1):
                hl = t0 // 8
                nc.sync.dma_start(
                    out=xt0_tiles[hl][:, t0 - 8 * hl:t1 - 8 * hl, :],
                    in_=xT_r[:, t0:t1, 0:512],
                )

            def w_piece(k0, k1):
                nc.sync.dma_start(
                    out=w_sb[:, k0:k1, 0:4 * HD],
                    in_=wq_r[:, k0:k1, 0:4 * HD],
                )

            xt0_piece(0, 1)
            w_piece(0, 1)
            w_piece(1, 4)
            xt0_piece(1, 4)
            w_piece(4, 8)
            xt0_piece(4, 8)
            w_piece(8, 12)
            xt0_piece(8, 12)
            w_piece(12, 16)
            xt0_piece(12, 16)
            xt_store[0] = xt0_tiles
            for half in range(2):
                nc.sync.dma_start(
                    out=w_sb[:, half * 8:(half + 1) * 8, 4 * HD:6 * HD],
                    in_=wq_r[:, half * 8:(half + 1) * 8, 4 * HD:6 * HD],
                )
            nc.sync.dma_start(
                out=bqk_sb[:], in_=bqk.ap().rearrange("(t p) o -> p (t o)", p=128)
            )
            load_xt(1)
            nc.vector.memset(ones_c[:], 1.0)
            # warmup matmuls: run during the startup DMA wait so the PE
            # HAM clock-gate is already at 8/8 when real work arrives.
            # bd_sb doubles as scratch: memset -> warm reads -> chunk-0 DMA
            # overwrites it (only consumed in section D).
            nc.vector.memset(bd_sb[:, 0:512], 1.0)
            wps = pp.tile([128, 512], f32, tag="work", bufs=2, name="warmps")
            for wi in range(10):
                nc.tensor.matmul(
                    wps[:], lhsT=bd_sb[:, 0:128], rhs=bd_sb[:, 0:512],
                    start=(wi == 0), stop=(wi == 9),
                )

            # ---- phase-1 chunk emission (generator; ~4-MM steps) ----
            def p1_chunk_steps(ci):
                b, nch = divmod(ci, 4)
                qk_sb, v_sb = qk_sbs[b], v_sbs[b]
                ch = slice(nch * 512, (nch + 1) * 512)
                if ci + 2 < 2 * (L // 512):
                    load_xt(ci + 2)
                if ci == 0:
                    nc.sync.dma_start(out=cos_sb[:], in_=cosT.ap())
                    nc.sync.dma_start(out=sin_sb[:], in_=sinT.ap())
                    nc.sync.dma_start(out=mask_sb[:], in_=mask0.ap())
                    nc.sync.dma_start(out=bd_sb[:], in_=bdb.ap())
                xt_tiles = xt_store.pop(ci)
                for mp in range(2):
                    ps2 = pp.tile([128, 1024], f32, tag="work", bufs=2,
                                  name=f"qkps{ci}_{mp}")
                    for kt in range(16):
                        xt = xt_tiles[kt // 8]
                        for i in range(2):
                            m = 2 * mp + i
                            nc.tensor.matmul(
                                ps2[:, i * 512:(i + 1) * 512],
                                lhsT=w_sb[:, kt, m * 128:(m + 1) * 128],
                                rhs=xt[:, kt % 8, :],
                                start=(kt == 0),
                                stop=(kt == 15),
                            )
                        if kt % 2 == 1:
                            yield
                    for i in range(2):
                        m = 2 * mp + i
                        nc.scalar.activation(
                            qk_sb[:, m, ch], ps2[:, i * 512:(i + 1) * 512],
                            AFT.Identity, bias=bqk_sb[:, m:m + 1],
                        )
                    yield
                for m in range(4):
                    # fused RoPE on rows 0:R (shuffle via DMA + 3 DVE ops)
                    cs = cos_sb[:, ch]
                    sn = sin_sb[:, ch]
                    rot = p1pool.tile([R, 512], f16, tag="rot", bufs=1)
                    nc.sync.dma_start(out=rot[0:16, :], in_=qk_sb[16:32, m, ch])
                    nc.sync.dma_start(out=rot[16:32, :], in_=qk_sb[0:16, m, ch])
                    nc.vector.tensor_tensor(
                        qk_sb[0:R, m, ch], qk_sb[0:R, m, ch], cs, op=OP.mult
                    )
                    nc.vector.tensor_tensor(rot[:], rot[:], sn, op=OP.mult)
                    nc.vector.tensor_tensor(
                        qk_sb[0:R, m, ch], qk_sb[0:R, m, ch], rot[:], op=OP.add
                    )
                    if m % 2 == 1:
                        yield
                for rr2 in range(2):
                    vpss = [
                        pp.tile([128, 2 * HD], f32,
                                tag=("work" if i else "acc"),
                                bufs=(2 if i else 3),
                                name=f"vps{ci}_{2 * rr2 + i}")
                        for i in range(2)
                    ]
                    for kt in range(16):
                        xt = xt_tiles[kt // 8]
                        for i in range(2):
                            rr = 2 * rr2 + i
                            nc.tensor.matmul(
                                vpss[i][:],
                                lhsT=xt[:, kt % 8, rr * 128:(rr + 1) * 128],
                                rhs=w_sb[:, kt, 4 * HD:6 * HD],
                                start=(kt == 0),
                                stop=(kt == 15),
                            )
                        if kt % 2 == 1:
                            yield
                    for i in range(2):
                        rr = 2 * rr2 + i
                        nc.scalar.activation(
                            v_sb[:, nch * 4 + rr, :], vpss[i][:], AFT.Copy
                        )
                    yield
                # stream Wd behind the x tiles
                nc.sync.dma_start(
                    out=wd_sb[:, 2 * ci:2 * (ci + 1), :],
                    in_=wd_r[:, 2 * ci:2 * (ci + 1), :],
                )

            # ---- attention emission (generator; yields per pair) ----
            last_ot = {}

            def attn_steps(b):
                qk_sb, v_sb = qk_sbs[b], v_sbs[b]
                for h in range(HPC):

                    def norm_a(state, b=b, h=h):
                        qc_p, outp_p, sumacc_p = state[:3]
                        sump = pp.tile([1, 512], f32, tag="sump", bufs=1,
                                       name=f"sump{b}_{h}_{qc_p}")
                        nc.tensor.matmul(
                            sump[:], lhsT=ones_c[:], rhs=sumacc_p[:],
                            start=True, stop=True,
                        )
                        rec1 = apool.tile([1, 512], f32, tag="bcs", bufs=1)
                        nc.vector.reciprocal_approx_fast(rec1[:], sump[:])
                        bcs = apool.tile([128, 512], f32, tag="rcp", bufs=1)
                        nc.gpsimd.partition_broadcast(bcs[:], rec1[:])
                        state.append(bcs)

                    def norm_b(state, b=b, h=h):
                        qc_p, outp_p, _, bcs = state
                        ot = apool.tile([128, 512], f16, tag="ot", bufs=2)
                        nc.vector.tensor_tensor(
                            ot[:], outp_p[:], bcs[:], op=OP.mult
                        )
                        for half in range(2):
                            last_ot[(b, h)] = nc.sync.dma_start(
                                out=a2a_ins[b][h][2 * qc_p + half, :, :],
                                in_=ot[:, half * 256:(half + 1) * 256],
                            )

                    prev = None   # [qc, outp, sumacc, (bcs)]
                    for qc in reversed(range(L // 512)):
                        nk = 4 * qc + 4
                        npairs = nk // 2
                        outp = pp.tile([128, 512], f32, tag="acc", bufs=3,
                                       name=f"outp{b}_{h}_{qc}")
                        sumacc = apool.tile([128, 512], f16, tag="sumacc",
                                            bufs=2)

                        def emit_out(pr, outp=outp, nk=nk, v_sb=v_sb, h=h):
                            ka, c0a, npra, kb, c0b, nprb, off, et2 = pr
                            nc.tensor.matmul(
                                outp[:, c0a:512],
                                lhsT=v_sb[:, ka, h * 128:(h + 1) * 128],
                                rhs=et2[:, 0:npra],
                                start=(ka == 0), stop=False,
                            )
                            nc.tensor.matmul(
                                outp[:, c0b:512],
                                lhsT=v_sb[:, kb, h * 128:(h + 1) * 128],
                                rhs=et2[:, off:off + nprb],
                                start=False, stop=(kb == nk - 1),
                            )

                        pend = []
                        for p in range(npairs):
                            ka, kb = 2 * p, 2 * p + 1
                            ja = max(0, ka - qc * 4)
                            jb = max(0, kb - qc * 4)
                            c0a, c0b = ja * 128, jb * 128
                            npra, nprb = 512 - c0a, 512 - c0b
                            off = npra
                            tot = npra + nprb
                            sp2 = pp.tile([128, 1024], f32, tag="work",
                                          bufs=2, name=f"sp{b}_{h}_{qc}_{p}")
                            nc.tensor.matmul(
                                sp2[:, 0:npra],
                                lhsT=qk_sb[:, 2 * h + 1,
                                           ka * 128:(ka + 1) * 128],
                                rhs=qk_sb[:, 2 * h,
                                          qc * 512 + c0a:(qc + 1) * 512],
                                start=True, stop=True,
                            )
                            nc.tensor.matmul(
                                sp2[:, off:off + nprb],
                                lhsT=qk_sb[:, 2 * h + 1,
                                           kb * 128:(kb + 1) * 128],
                                rhs=qk_sb[:, 2 * h,
                                          qc * 512 + c0b:(qc + 1) * 512],
                                start=True, stop=True,
                            )
                            et2 = apool.tile([128, 1024], f16, tag="et",
                                             bufs=4)
                            nc.scalar.activation(
                                et2[:, 0:tot], sp2[:, 0:tot], AFT.Exp,
                                scale=SCALE,
                            )
                            if ka >= qc * 4:
                                nc.vector.tensor_tensor(
                                    et2[:, 0:128], et2[:, 0:128], mask_sb[:],
                                    op=OP.mult,
                                )
                            if kb >= qc * 4:
                                nc.vector.tensor_tensor(
                                    et2[:, off:off + 128],
                                    et2[:, off:off + 128], mask_sb[:],
                                    op=OP.mult,
                                )
                            if p == 0:
                                # first tile always spans all 512 q-cols
                                nc.vector.tensor_copy(
                                    sumacc[:], et2[:, 0:512]
                                )
                            else:
                                nc.vector.tensor_tensor(
                                    sumacc[:, c0a:512], sumacc[:, c0a:512],
                                    et2[:, 0:npra], op=OP.add,
                                )
                            nc.vector.tensor_tensor(
                                sumacc[:, c0b:512], sumacc[:, c0b:512],
                                et2[:, off:off + nprb], op=OP.add,
                            )
                            pend.append(
                                (ka, c0a, npra, kb, c0b, nprb, off, et2)
                            )
                            if len(pend) > 2:
                                emit_out(pend.pop(0))
                            if p == 1 and prev is not None:
                                norm_a(prev)
                            if p == 3 and prev is not None:
                                norm_b(prev)
                                prev = None
                            yield
                        for pr in pend:
                            emit_out(pr)
                            yield
                        if prev is not None:   # qc with only 2 pairs
                            norm_b(prev)
                        prev = [qc, outp, sumacc]
                    norm_a(prev)
                    norm_b(prev)
                    prev = None
                    nc.gpsimd.collective_compute(
                        "AllToAll",
                        mybir.AluOpType.bypass,
                        replica_groups=[CORES],
                        ins=[a2a_ins[b][h][:]],
                        outs=[a2a_outs[b][h][:]],
                    )
                    yield

            # ---- o_sb reshard load groups ----
            o_sbs = [None, None]
            osb_chain = [None]

            def emit_osb_group(b, h, gate, engine):
                if o_sbs[b] is None:
                    o_sbs[b] = qkvpool.tile([128, 16, 256], f16, tag="shr",
                                            bufs=2, name=f"osb{b}")
                for jsrc in range(W):
                    ld = engine.dma_start(
                        out=o_sbs[b][:, 2 * jsrc + h, :],
                        in_=a2a_outs[b][h][jsrc, :, :],
                    )
                    if gate is not None:
                        _add_dep_helper(
                            ld.ins, gate.ins, sync=True,
                            reason="order o_sb after time-critical DMAs",
                        )
                    if engine is nc.sync:
                        if osb_chain[0] is not None:
                            _add_dep_helper(
                                ld.ins, osb_chain[0].ins, sync=True,
                                reason="FIFO-chain o_sb loads",
                            )
                        osb_chain[0] = ld

            # ---- phase-3 emission (generator; yields per half-chain) ----
            def p3_steps(bh):
                for n4 in range(4):
                    for i in range(2):
                        m = 2 * bh + i
                        yp = pp.tile([128, 512], f32, tag="work", bufs=2,
                                     name=f"yps{bh}_{n4}_{m}")
                        for kt in range(16):
                            nc.tensor.matmul(
                                yp[:],
                                lhsT=o_sbs[bh][:, kt, i * 128:(i + 1) * 128],
                                rhs=wd_sb[:, kt, n4 * 512:(n4 + 1) * 512],
                                start=(kt == 0), stop=(kt == 15),
                            )
                            if kt == 7:
                                yield
                        yt = p3s.tile([128, 512], f16, tag="yt", bufs=2)
                        nc.vector.tensor_tensor(
                            yt[:], yp[:], bd_sb[:, n4 * 512:(n4 + 1) * 512],
                            op=OP.add,
                        )
                        nc.sync.dma_start(
                            out=y[m * 128:(m + 1) * 128,
                                  n4 * 512:(n4 + 1) * 512],
                            in_=yt[:],
                        )
                        yield

            def drain(gen):
                for _ in gen:
                    pass

            def pull(gen, n):
                for _ in range(n):
                    if next(gen, StopIteration) is StopIteration:
                        return False
                return True

            # ======== section A: qkv projection b0 ========
            for ci in range(4):
                drain(p1_chunk_steps(ci))

            # ======== section B: attention(b0) x qkv projection(b1) ======
            def p1_rest():
                for ci in range(4, 8):
                    for _ in p1_chunk_steps(ci):
                        yield

            gA = attn_steps(0)
            gP = p1_rest()
            alive = True
            for _ in gA:
                if alive:
                    alive = pull(gP, 2)
            if alive:
                drain(gP)

            # o_sb loads for b0h0 (sync queue, gated after b0h1's ot writes)
            emit_osb_group(0, 0, last_ot[(0, 1)], nc.sync)

            # ======== section C: attention(b1) ========
            gA = attn_steps(1)
            pull(gA, 14)
            emit_osb_group(0, 1, last_ot.get((1, 0)), nc.sync)
            drain(gA)

            # o_sb loads for b1: h0 on sync (gated), h1 on the ACT hw queue
            emit_osb_group(1, 0, last_ot[(1, 1)], nc.sync)
            emit_osb_group(1, 1, None, nc.scalar)

            # ======== section D: out-projection (b0 hides the last A2A) ==
            drain(p3_steps(0))
            # phase3-b1 wave A: 4 chains split even/odd kt. Even kt = head-0
            # channels (ready after the 3rd A2A), so the PE keeps working
            # while the last A2A is still in flight.
            waveA = [(0, 0), (0, 1), (1, 0), (1, 1)]
            ypw = {}
            for idx, (n4, i) in enumerate(waveA):
                yp = pp.tile([128, 512], f32,
                             tag=("work" if idx < 2 else "acc"),
                             bufs=(2 if idx < 2 else 3),
                             name=f"ypw{n4}_{i}")
                for kt in range(0, 16, 2):
                    nc.tensor.matmul(
                        yp[:],
                        lhsT=o_sbs[1][:, kt, i * 128:(i + 1) * 128],
                        rhs=wd_sb[:, kt, n4 * 512:(n4 + 1) * 512],
                        start=(kt == 0), stop=False,
                    )
                ypw[(n4, i)] = yp
            for (n4, i) in waveA:
                yp = ypw[(n4, i)]
                m = 2 + i
                for kt in range(1, 16, 2):
                    nc.tensor.matmul(
                        yp[:],
                        lhsT=o_sbs[1][:, kt, i * 128:(i + 1) * 128],
                        rhs=wd_sb[:, kt, n4 * 512:(n4 + 1) * 512],
                        start=False, stop=(kt == 15),
                    )
                yt = p3s.tile([128, 512], f16, tag="yt", bufs=2)
                nc.vector.tensor_tensor(
                    yt[:], yp[:], bd_sb[:, n4 * 512:(n4 + 1) * 512],
                    op=OP.add,
                )
                nc.sync.dma_start(
                    out=y[m * 128:(m + 1) * 128, n4 * 512:(n4 + 1) * 512],
                    in_=yt[:],
                )
            # wave B: remaining 4 chains, contiguous
            for n4 in (2, 3):
                for i in range(2):
                    m = 2 + i
                    yp = pp.tile([128, 512], f32, tag="work", bufs=2,
                                 name=f"ypb{n4}_{i}")
                    for kt in range(16):
                        nc.tensor.matmul(
                            yp[:],
                            lhsT=o_sbs[1][:, kt, i * 128:(i + 1) * 128],
                            rhs=wd_sb[:, kt, n4 * 512:(n4 + 1) * 512],
                            start=(kt == 0), stop=(kt == 15),
                        )
                    yt = p3s.tile([128, 512], f16, tag="yt", bufs=2)
                    nc.vector.tensor_tensor(
                        yt[:], yp[:], bd_sb[:, n4 * 512:(n4 + 1) * 512],
                        op=OP.add,
                    )
                    nc.sync.dma_start(
                        out=y[m * 128:(m + 1) * 128,
                              n4 * 512:(n4 + 1) * 512],
                        in_=yt[:],
                    )
    nc.finalize()
    return nc


def _host_prep(x_BLD, cos, sin, Wqkv, bqkv, Wd, bd):
    x = np.asarray(x_BLD, np.float32).reshape(M, D)
    xT = np.ascontiguousarray(x.T.astype(np.float16))
    cosT = np.ascontiguousarray(
        np.asarray(cos, np.float32).reshape(L, R).T.astype(np.float16)
    )
    s2 = np.asarray(sin, np.float32).reshape(L, R).T
    sinT_pm = np.ascontiguousarray(
        np.concatenate([-s2[:16], s2[16:]], axis=0).astype(np.float16)
    )
    kk = np.arange(128, dtype=np.int64)[:, None]
    qq = np.arange(128, dtype=np.int64)[None, :]
    mask0 = np.ascontiguousarray((qq >= kk).astype(np.float16))
    Wqkv = np.asarray(Wqkv, np.float32)
    bqkv = np.asarray(bqkv, np.float32)
    # softmax rows sum to 1, so the v-bias contribution collapses to a
    # constant vector bv_glob @ Wd folded into bd
    HDl = D // H
    v_bias = np.concatenate(
        [bqkv[hh * 3 * HDl + 2 * HDl:(hh * 3 + 3) * HDl] for hh in range(H)]
    )
    bd_eff = np.asarray(bd, np.float32) + v_bias @ np.asarray(Wd, np.float32)
    bdb = np.ascontiguousarray(
        np.broadcast_to(bd_eff.astype(np.float16), (128, D))
    )
    in_maps = []
    for c in range(W):
        base = c * HPC * 3 * HD
        qk_idx = np.concatenate(
            [np.arange(base + h * 3 * HD, base + h * 3 * HD + 2 * HD)
             for h in range(HPC)]
        )
        v_idx = np.concatenate(
            [np.arange(base + h * 3 * HD + 2 * HD, base + (h + 1) * 3 * HD)
             for h in range(HPC)]
        )
        in_maps.append({
            "xT": xT,
            "wqkv": np.ascontiguousarray(
                Wqkv[:, np.concatenate([qk_idx, v_idx])].astype(np.float16)
            ),
            "bqk": np.ascontiguousarray(bqkv[qk_idx].reshape(4 * HD, 1)),
            "cosT": cosT,
            "sinT": sinT_pm,
            "mask0": mask0,
            "wd": np.asarray(Wd, np.float32).astype(np.float16),
            "bdb": bdb,
        })
    return in_maps


def _get_nc():
    global _NC
    if _NC is None:
        _NC = _build_nc()
    return _NC


def _run(inputs, trace=False, tmpdir=None):
    from concourse.bass_utils import run_bass_kernel_spmd

    in_maps = _host_prep(**inputs)
    nc = _get_nc()
    res = run_bass_kernel_spmd(nc, in_maps, CORES, trace=trace, tmpdir=tmpdir)
    out = np.empty((M, D), np.float32)
    for c in CORES:
        yc = np.asarray(res.results[c]["y"], np.float32)  # [512, D]
        out[c * 256:(c + 1) * 256] = yc[:256]
        out[L + c * 256:L + (c + 1) * 256] = yc[256:]
    return out.reshape(B, L, D), res


def kernel(**inputs) -> np.ndarray:
    out, _ = _run(inputs)
    return out



# revision 14
# speedup vs baseline: 1.0884x; 1.0286x over previous
"""Distributed Trainium2 kernel for nn_DecoderAttentionRotary.

Strategy (8 NeuronCores, tensor-parallel over heads, fp16 matmul datapath):
  - host: transpose x -> xT [D, B*L] fp16; per-core Wqkv column slice
    reordered to [q0,k0,q1,k1,v0|v1] fp16; cos/sin transposed fp16; one
    128x128 causal mask; v-bias folded into bd (softmax rows sum to 1,
    so attn@(xWv+1*bv)@Wd = attn@xWv@Wd + bv@Wd).
  - device, per core (2 heads), with CROSS-PHASE INTERLEAVED emission
    (attention is ACT/exp-throughput-bound, projections are PE-bound):
      section A: qkv projection b0 (startup DMAs finely interleaved,
                 v-columns of W deferred),
      section B: attention(b0) interleaved ~1:2 with qkv projection(b1);
                 the b0 AllToAlls are fully hidden here,
      section C: attention(b1) compressed (fast A2As),
      section D: out-projection; b0's half hides the last A2A, and the
                 first 4 chains of b1's half are split even/odd k so the
                 even (head-0) halves also run before the last A2A lands.
  - attention: scores^T layout; k-tiles in PAIRS sharing one [128,1024]
    PSUM tile and ONE packed exp (amortizes the ~260ns ACT instruction
    overhead); out-matmuls lag 2 pairs behind scores; row-sums
    accumulated on DVE (sumacc += et), reduced by a single ones-matmul
    per q-chunk + fast approximate reciprocal + gpsimd broadcast, all
    staggered one q-chunk behind compute so nothing waits on the chain.
  - per-(batch,head) AllToAll reshard (fp16); o_sb reshard loads are
    FIFO-chained plain copies explicitly dep-ordered after the next
    head's attention-output DMAs (the tile scheduler otherwise hoists
    them where their semaphore waits head-of-line-block the sync queue);
    the final group is issued from the ACT hardware DMA queue.
  - y computed in fp16 (halves output DMA), upcast to fp32 on host.
  - host: scatter the per-core 256-row halves into the full output.
"""
import sys

for _p in ("/opt/pypackages", "/opt/trn_rl_repo"):
    if _p not in sys.path:
        sys.path.insert(0, _p)

import numpy as np

B, L, D, H = 2, 2048, 2048, 16
HD, R = 128, 32
SCALE = float(HD) ** -0.5
W = 8
HPC = H // W              # heads per core
M = B * L                 # flattened rows
CORES = list(range(W))

_NC = None


def _build_nc():
    import concourse.mybir as mybir
    import concourse.tile as tile
    from concourse import bacc
    from concourse.bass import _add_dep_helper
    from concourse.bass_isa import ReduceOp

    f32 = mybir.dt.float32
    f16 = mybir.dt.float16
    AFT = mybir.ActivationFunctionType
    OP = mybir.AluOpType

    nc = bacc.Bacc(None, target_bir_lowering=False, num_devices=W)
    xT = nc.declare_dram_parameter("xT", [D, M], f16, isOutput=False)
    wqkv = nc.declare_dram_parameter("wqkv", [D, 6 * HD], f16, isOutput=False)
    bqk = nc.declare_dram_parameter("bqk", [4 * HD, 1], f32, isOutput=False)
    cosT = nc.declare_dram_parameter("cosT", [R, L], f16, isOutput=False)
    sinT = nc.declare_dram_parameter("sinT", [R, L], f16, isOutput=False)
    mask0 = nc.declare_dram_parameter("mask0", [128, 128], f16, isOutput=False)
    wd = nc.declare_dram_parameter("wd", [D, D], f16, isOutput=False)
    bdb = nc.declare_dram_parameter("bdb", [128, D], f16, isOutput=False)
    y = nc.declare_dram_parameter("y", [M // W, D], f16, isOutput=True)

    xT_r = xT.ap().rearrange("(t p) n -> p t n", p=128)   # [128, 16, M]
    wq_r = wqkv.ap().rearrange("(t p) m -> p t m", p=128)
    wd_r = wd.ap().rearrange("(t p) n -> p t n", p=128)

    with tile.TileContext(nc) as tc:
        with (
            tc.tile_pool(name="const", bufs=1) as cpool,
            tc.tile_pool(name="dram", bufs=1, space="DRAM") as dpool,
            tc.tile_pool(name="ps", bufs=1, space="PSUM") as pp,
            tc.tile_pool(name="qkv", bufs=1) as qkvpool,
            tc.tile_pool(name="p3s", bufs=2) as p3s,
            tc.tile_pool(name="att", bufs=2) as apool,
            tc.tile_pool(name="p1", bufs=2) as p1pool,
        ):
            a2a_ins = [[dpool.tile([W, HD, 256], f16, name=f"a2ain{b}_{h}")
                        for h in range(HPC)] for b in range(B)]
            a2a_outs = [[dpool.tile([W, HD, 256], f16, name=f"a2aout{b}_{h}")
                         for h in range(HPC)] for b in range(B)]

            # w_sb (qkv weights, live in A/B) and wd_sb (out-proj weights,
            # live in C/D) share one 64KB slot; wd's deferred DMAs WAR-wait
            # on w's last read automatically.
            w_sb = cpool.tile([128, 16, 6 * HD], f16, tag="wbig", bufs=1,
                              padded_shape=[128, 16, D])
            wd_sb = cpool.tile([128, 16, D], f16, tag="wbig", bufs=1)
            bd_sb = cpool.tile([128, D], f16)
            bqk_sb = cpool.tile([128, 4], f32)
            ones_c = cpool.tile([128, 1], f16)
            cos_sb = qkvpool.tile([R, L], f16, tag="shr", bufs=2,
                                  padded_shape=[128, 16 * 256])
            sin_sb = qkvpool.tile([R, L], f16, tag="shr", bufs=2,
                                  padded_shape=[128, 16 * 256])
            mask_sb = cpool.tile([128, 128], f16)
            qk_sbs, v_sbs = [], []
            for b in range(B):
                qk_sbs.append(qkvpool.tile([128, 4, L], f16, name=f"qk{b}"))
                v_sbs.append(qkvpool.tile([128, 16, 2 * HD], f16, name=f"v{b}"))

            # ---- startup ----
            # sync queue carries ONLY xt pieces (big, latency-critical);
            # weights/consts go on the scalar queue so neither FIFO
            # head-blocks the other.  xt double-buffer depth is 2 full
            # chunks (bufs=4) so a chunk's loads never WAR-wait on the
            # previous chunk's reads and always land early.
            xt_store = {}

            def load_xt(ci):
                b, nch = divmod(ci, 4)
                n0 = b * L + nch * 512
                tiles = []
                for half in range(2):
                    xt = p1pool.tile([128, 8, 512], f16, tag="xt", bufs=4,
                                     name=f"xt{ci}_{half}")
                    for piece in range(2):
                        t0 = half * 8 + piece * 4
                        nc.sync.dma_start(
                            out=xt[:, piece * 4:(piece + 1) * 4, :],
                            in_=xT_r[:, t0:t0 + 4, n0:n0 + 512],
                        )
                    tiles.append(xt)
                xt_store[ci] = tiles

            xt0_tiles = []
            for half in range(2):
                xt = p1pool.tile([128, 8, 512], f16, tag="xt", bufs=4,
                                 name=f"xt0_{half}")
                xt0_tiles.append(xt)
            # startup pieces interleaved on sync, consumption-ordered;
            # first pieces tiny so the first matmul can start ASAP
            def xt0_piece(t0, t1):
                hl = t0 // 8
                nc.sync.dma_start(
                    out=xt0_tiles[hl][:, t0 - 8 * hl:t1 - 8 * hl, :],
                    in_=xT_r[:, t0:t1, 0:512],
                )

            def w_piece(k0, k1):
                nc.sync.dma_start(
                    out=w_sb[:, k0:k1, 0:4 * HD],
                    in_=wq_r[:, k0:k1, 0:4 * HD],
                )

            xt0_piece(0, 1)
            w_piece(0, 1)
            w_piece(1, 4)
            xt0_piece(1, 4)
            w_piece(4, 8)
            xt0_piece(4, 8)
            w_piece(8, 12)
            xt0_piece(8, 12)
            w_piece(12, 16)
            xt0_piece(12, 16)
            xt_store[0] = xt0_tiles
            for half in range(2):
                nc.sync.dma_start(
                    out=w_sb[:, half * 8:(half + 1) * 8, 4 * HD:6 * HD],
                    in_=wq_r[:, half * 8:(half + 1) * 8, 4 * HD:6 * HD],
                )
            nc.sync.dma_start(
                out=bqk_sb[:], in_=bqk.ap().rearrange("(t p) o -> p (t o)", p=128)
            )
            load_xt(1)
            nc.vector.memset(ones_c[:], 1.0)
            # warmup matmuls: run during the startup DMA wait so the PE
            # HAM clock-gate is already at 8/8 when real work arrives.
            # bd_sb doubles as scratch: memset -> warm reads -> chunk-0 DMA
            # overwrites it (only consumed in section D).
            nc.vector.memset(bd_sb[:, 0:512], 1.0)
            wps = pp.tile([128, 512], f32, tag="work", bufs=2, name="warmps")
            for wi in range(10):
                nc.tensor.matmul(
                    wps[:], lhsT=bd_sb[:, 0:128], rhs=bd_sb[:, 0:512],
                    start=(wi == 0), stop=(wi == 9),
                )

            # ---- phase-1 chunk emission (generator; ~4-MM steps) ----
            def p1_chunk_steps(ci):
                b, nch = divmod(ci, 4)
                qk_sb, v_sb = qk_sbs[b], v_sbs[b]
                ch = slice(nch * 512, (nch + 1) * 512)
                if ci + 2 < 2 * (L // 512):
                    load_xt(ci + 2)
                if ci == 0:
                    nc.sync.dma_start(out=cos_sb[:], in_=cosT.ap())
                    nc.sync.dma_start(out=sin_sb[:], in_=sinT.ap())
                    nc.sync.dma_start(out=mask_sb[:], in_=mask0.ap())
                    nc.sync.dma_start(out=bd_sb[:], in_=bdb.ap())
                xt_tiles = xt_store.pop(ci)
                for mp in range(2):
                    ps2 = pp.tile([128, 1024], f32, tag="work", bufs=2,
                                  name=f"qkps{ci}_{mp}")
                    for kt in range(16):
                        xt = xt_tiles[kt // 8]
                        for i in range(2):
                            m = 2 * mp + i
                            nc.tensor.matmul(
                                ps2[:, i * 512:(i + 1) * 512],
                                lhsT=w_sb[:, kt, m * 128:(m + 1) * 128],
                                rhs=xt[:, kt % 8, :],
                                start=(kt == 0),
                                stop=(kt == 15),
                            )
                        if kt % 2 == 1:
                            yield
                    for i in range(2):
                        m = 2 * mp + i
                        nc.scalar.activation(
                            qk_sb[:, m, ch], ps2[:, i * 512:(i + 1) * 512],
                            AFT.Identity, bias=bqk_sb[:, m:m + 1],
                        )
                    yield
                for m in range(4):
                    # fused RoPE on rows 0:R (shuffle via DMA + 3 DVE ops)
                    cs = cos_sb[:, ch]
                    sn = sin_sb[:, ch]
                    rot = p1pool.tile([R, 512], f16, tag="rot", bufs=2)
                    nc.sync.dma_start(out=rot[0:16, :], in_=qk_sb[16:32, m, ch])
                    nc.sync.dma_start(out=rot[16:32, :], in_=qk_sb[0:16, m, ch])
                    nc.vector.tensor_tensor(
                        qk_sb[0:R, m, ch], qk_sb[0:R, m, ch], cs, op=OP.mult
                    )
                    nc.vector.tensor_tensor(rot[:], rot[:], sn, op=OP.mult)
                    nc.vector.tensor_tensor(
                        qk_sb[0:R, m, ch], qk_sb[0:R, m, ch], rot[:], op=OP.add
                    )
                    if m % 2 == 1:
                        yield
                for rr2 in range(2):
                    vpss = [
                        pp.tile([128, 2 * HD], f32,
                                tag=("work" if i else "acc"),
                                bufs=(2 if i else 3),
                                name=f"vps{ci}_{2 * rr2 + i}")
                        for i in range(2)
                    ]
                    for kt in range(16):
                        xt = xt_tiles[kt // 8]
                        for i in range(2):
                            rr = 2 * rr2 + i
                            nc.tensor.matmul(
                                vpss[i][:],
                                lhsT=xt[:, kt % 8, rr * 128:(rr + 1) * 128],
                                rhs=w_sb[:, kt, 4 * HD:6 * HD],
                                start=(kt == 0),
                                stop=(kt == 15),
                            )
                        if kt % 2 == 1:
                            yield
                    for i in range(2):
                        rr = 2 * rr2 + i
                        nc.scalar.activation(
                            v_sb[:, nch * 4 + rr, :], vpss[i][:], AFT.Copy
                        )
                    yield

            # ---- attention emission (generator; yields per pair) ----
            last_ot = {}

            def attn_steps(b):
                qk_sb, v_sb = qk_sbs[b], v_sbs[b]
                for h in range(HPC):

                    def norm_a(state, b=b, h=h):
                        qc_p, outp_p, sumacc_p = state[:3]
                        sump = pp.tile([1, 512], f32, tag="sump", bufs=1,
                                       name=f"sump{b}_{h}_{qc_p}")
                        nc.tensor.matmul(
                            sump[:], lhsT=ones_c[:], rhs=sumacc_p[:],
                            start=True, stop=True,
                        )
                        rec1 = apool.tile([1, 512], f32, tag="bcs", bufs=1)
                        nc.vector.reciprocal_approx_fast(rec1[:], sump[:])
                        bcs = apool.tile([128, 512], f32, tag="rcp", bufs=1)
                        nc.gpsimd.partition_broadcast(bcs[:], rec1[:])
                        state.append(bcs)

                    def norm_b(state, b=b, h=h):
                        qc_p, outp_p, _, bcs = state
                        ot = apool.tile([128, 512], f16, tag="ot", bufs=3)
                        nc.vector.tensor_tensor(
                            ot[:], outp_p[:], bcs[:], op=OP.mult
                        )
                        for half in range(2):
                            last_ot[(b, h)] = nc.sync.dma_start(
                                out=a2a_ins[b][h][2 * qc_p + half, :, :],
                                in_=ot[:, half * 256:(half + 1) * 256],
                            )

                    prev = None   # [qc, outp, sumacc, (bcs)]
                    for qc in reversed(range(L // 512)):
                        nk = 4 * qc + 4
                        npairs = nk // 2
                        outp = pp.tile([128, 512], f32, tag="acc", bufs=3,
                                       name=f"outp{b}_{h}_{qc}")
                        sumacc = apool.tile([128, 512], f16, tag="sumacc",
                                            bufs=2)

                        def emit_out(pr, outp=outp, nk=nk, v_sb=v_sb, h=h):
                            ka, c0a, npra, kb, c0b, nprb, off, et2 = pr
                            nc.tensor.matmul(
                                outp[:, c0a:512],
                                lhsT=v_sb[:, ka, h * 128:(h + 1) * 128],
                                rhs=et2[:, 0:npra],
                                start=(ka == 0), stop=False,
                            )
                            nc.tensor.matmul(
                                outp[:, c0b:512],
                                lhsT=v_sb[:, kb, h * 128:(h + 1) * 128],
                                rhs=et2[:, off:off + nprb],
                                start=False, stop=(kb == nk - 1),
                            )

                        pend = []
                        for p in range(npairs):
                            ka, kb = 2 * p, 2 * p + 1
                            ja = max(0, ka - qc * 4)
                            jb = max(0, kb - qc * 4)
                            c0a, c0b = ja * 128, jb * 128
                            npra, nprb = 512 - c0a, 512 - c0b
                            off = npra
                            tot = npra + nprb
                            sp2 = pp.tile([128, 1024], f32, tag="work",
                                          bufs=2, name=f"sp{b}_{h}_{qc}_{p}")
                            nc.tensor.matmul(
                                sp2[:, 0:npra],
                                lhsT=qk_sb[:, 2 * h + 1,
                                           ka * 128:(ka + 1) * 128],
                                rhs=qk_sb[:, 2 * h,
                                          qc * 512 + c0a:(qc + 1) * 512],
                                start=True, stop=True,
                            )
                            nc.tensor.matmul(
                                sp2[:, off:off + nprb],
                                lhsT=qk_sb[:, 2 * h + 1,
                                           kb * 128:(kb + 1) * 128],
                                rhs=qk_sb[:, 2 * h,
                                          qc * 512 + c0b:(qc + 1) * 512],
                                start=True, stop=True,
                            )
                            et2 = apool.tile([128, 1024], f16, tag="et",
                                             bufs=4)
                            nc.scalar.activation(
                                et2[:, 0:tot], sp2[:, 0:tot], AFT.Exp,
                                scale=SCALE,
                            )
                            if ka >= qc * 4:
                                nc.vector.tensor_tensor(
                                    et2[:, 0:128], et2[:, 0:128], mask_sb[:],
                                    op=OP.mult,
                                )
                            if kb >= qc * 4:
                                nc.vector.tensor_tensor(
                                    et2[:, off:off + 128],
                                    et2[:, off:off + 128], mask_sb[:],
                                    op=OP.mult,
                                )
                            if p == 0:
                                # first tile always spans all 512 q-cols
                                nc.vector.tensor_copy(
                                    sumacc[:], et2[:, 0:512]
                                )
                            else:
                                nc.vector.tensor_tensor(
                                    sumacc[:, c0a:512], sumacc[:, c0a:512],
                                    et2[:, 0:npra], op=OP.add,
                                )
                            nc.vector.tensor_tensor(
                                sumacc[:, c0b:512], sumacc[:, c0b:512],
                                et2[:, off:off + nprb], op=OP.add,
                            )
                            pend.append(
                                (ka, c0a, npra, kb, c0b, nprb, off, et2)
                            )
                            if len(pend) > 2:
                                emit_out(pend.pop(0))
                            if p == 1 and prev is not None:
                                norm_a(prev)
                            if p == 3 and prev is not None:
                                norm_b(prev)
                                prev = None
                            yield
                        for pr in pend:
                            emit_out(pr)
                            yield
                        if prev is not None:   # qc with only 2 pairs
                            norm_b(prev)
                        prev = [qc, outp, sumacc]
                    norm_a(prev)
                    norm_b(prev)
                    prev = None
                    nc.gpsimd.collective_compute(
                        "AllToAll",
                        mybir.AluOpType.bypass,
                        replica_groups=[CORES],
                        ins=[a2a_ins[b][h][:]],
                        outs=[a2a_outs[b][h][:]],
                    )
                    yield

            # ---- o_sb reshard load groups ----
            o_sbs = [None, None]
            osb_chain = [None]

            def emit_osb_group(b, h, gate, engine):
                if o_sbs[b] is None:
                    o_sbs[b] = qkvpool.tile([128, 16, 256], f16, tag="shr",
                                            bufs=2, name=f"osb{b}")
                for jsrc in range(W):
                    ld = engine.dma_start(
                        out=o_sbs[b][:, 2 * jsrc + h, :],
                        in_=a2a_outs[b][h][jsrc, :, :],
                    )
                    if gate is not None:
                        _add_dep_helper(
                            ld.ins, gate.ins, sync=True,
                            reason="order o_sb after time-critical DMAs",
                        )
                    if engine is nc.sync:
                        if osb_chain[0] is not None:
                            _add_dep_helper(
                                ld.ins, osb_chain[0].ins, sync=True,
                                reason="FIFO-chain o_sb loads",
                            )
                        osb_chain[0] = ld

            # ---- phase-3 emission (generator; yields per half-chain) ----
            def p3_steps(bh):
                for n4 in range(4):
                    for i in range(2):
                        m = 2 * bh + i
                        yp = pp.tile([128, 512], f32, tag="work", bufs=2,
                                     name=f"yps{bh}_{n4}_{m}")
                        for kt in range(16):
                            nc.tensor.matmul(
                                yp[:],
                                lhsT=o_sbs[bh][:, kt, i * 128:(i + 1) * 128],
                                rhs=wd_sb[:, kt, n4 * 512:(n4 + 1) * 512],
                                start=(kt == 0), stop=(kt == 15),
                            )
                            if kt == 7:
                                yield
                        yt = p3s.tile([128, 512], f16, tag="yt", bufs=4)
                        nc.vector.tensor_tensor(
                            yt[:], yp[:], bd_sb[:, n4 * 512:(n4 + 1) * 512],
                            op=OP.add,
                        )
                        nc.sync.dma_start(
                            out=y[m * 128:(m + 1) * 128,
                                  n4 * 512:(n4 + 1) * 512],
                            in_=yt[:],
                        )
                        yield

            def drain(gen):
                for _ in gen:
                    pass

            def pull(gen, n):
                for _ in range(n):
                    if next(gen, StopIteration) is StopIteration:
                        return False
                return True

            # ======== section A: qkv projection b0 ========
            for ci in range(4):
                drain(p1_chunk_steps(ci))

            # ======== section B: attention(b0) x qkv projection(b1) ======
            def p1_rest():
                for ci in range(4, 8):
                    for _ in p1_chunk_steps(ci):
                        yield

            gA = attn_steps(0)
            gP = p1_rest()
            alive = True
            for _ in gA:
                if alive:
                    alive = pull(gP, 2)
            if alive:
                drain(gP)

            # o_sb loads for b0h0 (sync queue, gated after b0h1's ot writes)
            emit_osb_group(0, 0, last_ot[(0, 1)], nc.sync)

            # wd loads deferred here: the shared slot frees when the last
            # qkv matmul of section B has read w_sb
            for wci in range(8):
                nc.sync.dma_start(
                    out=wd_sb[:, 2 * wci:2 * (wci + 1), :],
                    in_=wd_r[:, 2 * wci:2 * (wci + 1), :],
                )

            # ======== section C: attention(b1) ========
            gA = attn_steps(1)
            pull(gA, 14)
            emit_osb_group(0, 1, last_ot.get((1, 0)), nc.sync)
            drain(gA)

            # o_sb loads for b1: h0 on sync (gated), h1 on the ACT hw queue
            emit_osb_group(1, 0, last_ot[(1, 1)], nc.sync)
            emit_osb_group(1, 1, None, nc.scalar)

            # ======== section D: out-projection (b0 hides the last A2A) ==
            drain(p3_steps(0))
            # phase3-b1 wave A: 4 chains split even/odd kt. Even kt = head-0
            # channels (ready after the 3rd A2A), so the PE keeps working
            # while the last A2A is still in flight.
            waveA = [(0, 0), (0, 1), (1, 0), (1, 1)]
            ypw = {}
            for idx, (n4, i) in enumerate(waveA):
                yp = pp.tile([128, 512], f32,
                             tag=("work" if idx < 2 else "acc"),
                             bufs=(2 if idx < 2 else 3),
                             name=f"ypw{n4}_{i}")
                for kt in range(0, 16, 2):
                    nc.tensor.matmul(
                        yp[:],
                        lhsT=o_sbs[1][:, kt, i * 128:(i + 1) * 128],
                        rhs=wd_sb[:, kt, n4 * 512:(n4 + 1) * 512],
                        start=(kt == 0), stop=False,
                    )
                ypw[(n4, i)] = yp
            for (n4, i) in waveA:
                yp = ypw[(n4, i)]
                m = 2 + i
                for kt in range(1, 16, 2):
                    nc.tensor.matmul(
                        yp[:],
                        lhsT=o_sbs[1][:, kt, i * 128:(i + 1) * 128],
                        rhs=wd_sb[:, kt, n4 * 512:(n4 + 1) * 512],
                        start=False, stop=(kt == 15),
                    )
                yt = p3s.tile([128, 512], f16, tag="yt", bufs=4)
                nc.vector.tensor_tensor(
                    yt[:], yp[:], bd_sb[:, n4 * 512:(n4 + 1) * 512],
                    op=OP.add,
                )
                nc.sync.dma_start(
                    out=y[m * 128:(m + 1) * 128, n4 * 512:(n4 + 1) * 512],
                    in_=yt[:],
                )
            # wave B: remaining 4 chains, contiguous
            for n4 in (2, 3):
                for i in range(2):
                    m = 2 + i
                    yp = pp.tile([128, 512], f32, tag="work", bufs=2,
                                 name=f"ypb{n4}_{i}")
                    for kt in range(16):
                        nc.tensor.matmul(
                            yp[:],
                            lhsT=o_sbs[1][:, kt, i * 128:(i + 1) * 128],
                            rhs=wd_sb[:, kt, n4 * 512:(n4 + 1) * 512],
                            start=(kt == 0), stop=(kt == 15),
                        )
                    yt = p3s.tile([128, 512], f16, tag="yt", bufs=4)
                    nc.vector.tensor_tensor(
                        yt[:], yp[:], bd_sb[:, n4 * 512:(n4 + 1) * 512],
                        op=OP.add,
                    )
                    nc.sync.dma_start(
                        out=y[m * 128:(m + 1) * 128,
                              n4 * 512:(n4 + 1) * 512],
                        in_=yt[:],
                    )
    nc.finalize()
    return nc


def _host_prep(x_BLD, cos, sin, Wqkv, bqkv, Wd, bd):
    x = np.asarray(x_BLD, np.float32).reshape(M, D)
    xT = np.ascontiguousarray(x.T.astype(np.float16))
    cosT = np.ascontiguousarray(
        np.asarray(cos, np.float32).reshape(L, R).T.astype(np.float16)
    )
    s2 = np.asarray(sin, np.float32).reshape(L, R).T
    sinT_pm = np.ascontiguousarray(
        np.concatenate([-s2[:16], s2[16:]], axis=0).astype(np.float16)
    )
    kk = np.arange(128, dtype=np.int64)[:, None]
    qq = np.arange(128, dtype=np.int64)[None, :]
    mask0 = np.ascontiguousarray((qq >= kk).astype(np.float16))
    Wqkv = np.asarray(Wqkv, np.float32)
    bqkv = np.asarray(bqkv, np.float32)
    # softmax rows sum to 1, so the v-bias contribution collapses to a
    # constant vector bv_glob @ Wd folded into bd
    HDl = D // H
    v_bias = np.concatenate(
        [bqkv[hh * 3 * HDl + 2 * HDl:(hh * 3 + 3) * HDl] for hh in range(H)]
    )
    bd_eff = np.asarray(bd, np.float32) + v_bias @ np.asarray(Wd, np.float32)
    bdb = np.ascontiguousarray(
        np.broadcast_to(bd_eff.astype(np.float16), (128, D))
    )
    in_maps = []
    for c in range(W):
        base = c * HPC * 3 * HD
        qk_idx = np.concatenate(
            [np.arange(base + h * 3 * HD, base + h * 3 * HD + 2 * HD)
             for h in range(HPC)]
        )
        v_idx = np.concatenate(
            [np.arange(base + h * 3 * HD + 2 * HD, base + (h + 1) * 3 * HD)
             for h in range(HPC)]
        )
        in_maps.append({
            "xT": xT,
            "wqkv": np.ascontiguousarray(
                Wqkv[:, np.concatenate([qk_idx, v_idx])].astype(np.float16)
            ),
            "bqk": np.ascontiguousarray(bqkv[qk_idx].reshape(4 * HD, 1)),
            "cosT": cosT,
            "sinT": sinT_pm,
            "mask0": mask0,
            "wd": np.asarray(Wd, np.float32).astype(np.float16),
            "bdb": bdb,
        })
    return in_maps


def _get_nc():
    global _NC
    if _NC is None:
        _NC = _build_nc()
    return _NC


def _run(inputs, trace=False, tmpdir=None):
    from concourse.bass_utils import run_bass_kernel_spmd

    in_maps = _host_prep(**inputs)
    nc = _get_nc()
    res = run_bass_kernel_spmd(nc, in_maps, CORES, trace=trace, tmpdir=tmpdir)
    out = np.empty((M, D), np.float32)
    for c in CORES:
        yc = np.asarray(res.results[c]["y"], np.float32)  # [512, D]
        out[c * 256:(c + 1) * 256] = yc[:256]
        out[L + c * 256:L + (c + 1) * 256] = yc[256:]
    return out.reshape(B, L, D), res


def kernel(**inputs) -> np.ndarray:
    out, _ = _run(inputs)
    return out



# revision 17
# speedup vs baseline: 1.1364x; 1.0441x over previous
"""Distributed Trainium2 kernel for nn_DecoderAttentionRotary.

Strategy (8 NeuronCores, tensor-parallel over heads, fp16 matmul datapath):
  - host: transpose x -> xT [D, B*L] fp16; per-core Wqkv column slice
    reordered to [q0,k0,q1,k1,v0|v1] fp16; cos/sin transposed fp16; one
    128x128 causal mask; v-bias folded into bd (softmax rows sum to 1,
    so attn@(xWv+1*bv)@Wd = attn@xWv@Wd + bv@Wd).
  - device, per core (2 heads), with CROSS-PHASE INTERLEAVED emission
    (attention is ACT/exp-throughput-bound, projections are PE-bound):
      section A: qkv projection b0 (startup DMAs finely interleaved,
                 v-columns of W deferred),
      section B: attention(b0) interleaved ~1:2 with qkv projection(b1);
                 the b0 AllToAlls are fully hidden here,
      section C: attention(b1) compressed (fast A2As),
      section D: out-projection; b0's half hides the last A2A, and the
                 first 4 chains of b1's half are split even/odd k so the
                 even (head-0) halves also run before the last A2A lands.
  - attention: scores^T layout; k-tiles in PAIRS sharing one [128,1024]
    PSUM tile and ONE packed exp (amortizes the ~260ns ACT instruction
    overhead); out-matmuls lag 2 pairs behind scores; row-sums
    accumulated on DVE (sumacc += et), reduced by a single ones-matmul
    per q-chunk + fast approximate reciprocal + gpsimd broadcast, all
    staggered one q-chunk behind compute so nothing waits on the chain.
  - per-(batch,head) AllToAll reshard (fp16); o_sb reshard loads are
    FIFO-chained plain copies explicitly dep-ordered after the next
    head's attention-output DMAs (the tile scheduler otherwise hoists
    them where their semaphore waits head-of-line-block the sync queue);
    the final group is issued from the ACT hardware DMA queue.
  - y computed in fp16 (halves output DMA), upcast to fp32 on host.
  - host: scatter the per-core 256-row halves into the full output.
"""
import sys

for _p in ("/opt/pypackages", "/opt/trn_rl_repo"):
    if _p not in sys.path:
        sys.path.insert(0, _p)

import numpy as np

B, L, D, H = 2, 2048, 2048, 16
HD, R = 128, 32
SCALE = float(HD) ** -0.5
W = 8
HPC = H // W              # heads per core
M = B * L                 # flattened rows
CORES = list(range(W))

_NC = None


def _build_nc():
    import concourse.mybir as mybir
    import concourse.tile as tile
    from concourse import bacc
    from concourse.bass import _add_dep_helper
    from concourse.bass_isa import ReduceOp

    f32 = mybir.dt.float32
    f16 = mybir.dt.float16
    AFT = mybir.ActivationFunctionType
    OP = mybir.AluOpType

    nc = bacc.Bacc(None, target_bir_lowering=False, num_devices=W)
    xT = nc.declare_dram_parameter("xT", [D, M], f16, isOutput=False)
    wqkv = nc.declare_dram_parameter("wqkv", [D, 6 * HD], f16, isOutput=False)
    bqk = nc.declare_dram_parameter("bqk", [4 * HD, 1], f32, isOutput=False)
    cosT = nc.declare_dram_parameter("cosT", [R, L], f16, isOutput=False)
    sinT = nc.declare_dram_parameter("sinT", [R, L], f16, isOutput=False)
    mask0 = nc.declare_dram_parameter("mask0", [128, 128], f16, isOutput=False)
    wd = nc.declare_dram_parameter("wd", [D, D], f16, isOutput=False)
    bdb = nc.declare_dram_parameter("bdb", [128, D], f16, isOutput=False)
    y = nc.declare_dram_parameter("y", [M // W, D], f16, isOutput=True)

    xT_r = xT.ap().rearrange("(t p) n -> p t n", p=128)   # [128, 16, M]
    wq_r = wqkv.ap().rearrange("(t p) m -> p t m", p=128)
    wd_r = wd.ap().rearrange("(t p) n -> p t n", p=128)

    with tile.TileContext(nc) as tc:
        with (
            tc.tile_pool(name="const", bufs=1) as cpool,
            tc.tile_pool(name="dram", bufs=1, space="DRAM") as dpool,
            tc.tile_pool(name="ps", bufs=1, space="PSUM") as pp,
            tc.tile_pool(name="qkv", bufs=1) as qkvpool,
            tc.tile_pool(name="p3s", bufs=2) as p3s,
            tc.tile_pool(name="att", bufs=2) as apool,
            tc.tile_pool(name="p1", bufs=2) as p1pool,
        ):
            a2a_ins = [[dpool.tile([W, HD, 256], f16, name=f"a2ain{b}_{h}")
                        for h in range(HPC)] for b in range(B)]
            a2a_outs = [[dpool.tile([W, HD, 256], f16, name=f"a2aout{b}_{h}")
                         for h in range(HPC)] for b in range(B)]

            w_sb = cpool.tile([128, 16, 6 * HD], f16)
            wd_sb = cpool.tile([128, 16, D], f16)
            bd_sb = cpool.tile([128, D], f16)
            bqk_sb = cpool.tile([128, 4], f32)
            ones_c = cpool.tile([128, 1], f16)
            cos_sb = qkvpool.tile([R, L], f16, tag="shr", bufs=2,
                                  padded_shape=[128, 16 * 256])
            sin_sb = qkvpool.tile([R, L], f16, tag="shr", bufs=2,
                                  padded_shape=[128, 16 * 256])
            mask_sb = cpool.tile([128, 128], f16)
            qk_sbs, v_sbs = [], []
            for b in range(B):
                qk_sbs.append(qkvpool.tile([128, 4, L], f16, name=f"qk{b}"))
                v_sbs.append(qkvpool.tile([128, 16, 2 * HD], f16, name=f"v{b}"))

            # ---- startup ----
            # sync queue carries ONLY xt pieces (big, latency-critical);
            # weights/consts go on the scalar queue so neither FIFO
            # head-blocks the other.  xt double-buffer depth is 2 full
            # chunks (bufs=4) so a chunk's loads never WAR-wait on the
            # previous chunk's reads and always land early.
            xt_store = {}

            def load_xt(ci):
                b, nch = divmod(ci, 4)
                n0 = b * L + nch * 512
                tiles = []
                for half in range(2):
                    xt = p1pool.tile([128, 8, 512], f16, tag="xt", bufs=4,
                                     name=f"xt{ci}_{half}")
                    for piece in range(2):
                        t0 = half * 8 + piece * 4
                        nc.sync.dma_start(
                            out=xt[:, piece * 4:(piece + 1) * 4, :],
                            in_=xT_r[:, t0:t0 + 4, n0:n0 + 512],
                        )
                    tiles.append(xt)
                xt_store[ci] = tiles

            xt0_tiles = []
            for half in range(2):
                xt = p1pool.tile([128, 8, 512], f16, tag="xt", bufs=4,
                                 name=f"xt0_{half}")
                xt0_tiles.append(xt)
            # startup pieces interleaved on sync, consumption-ordered;
            # first pieces tiny so the first matmul can start ASAP
            def xt0_piece(t0, t1):
                hl = t0 // 8
                nc.sync.dma_start(
                    out=xt0_tiles[hl][:, t0 - 8 * hl:t1 - 8 * hl, :],
                    in_=xT_r[:, t0:t1, 0:512],
                )

            def w_piece(k0, k1):
                nc.sync.dma_start(
                    out=w_sb[:, k0:k1, 0:4 * HD],
                    in_=wq_r[:, k0:k1, 0:4 * HD],
                )

            xt0_piece(0, 1)
            w_piece(0, 1)
            w_piece(1, 4)
            xt0_piece(1, 4)
            w_piece(4, 8)
            xt0_piece(4, 8)
            w_piece(8, 12)
            xt0_piece(8, 12)
            w_piece(12, 16)
            xt0_piece(12, 16)
            xt_store[0] = xt0_tiles
            for half in range(2):
                nc.sync.dma_start(
                    out=w_sb[:, half * 8:(half + 1) * 8, 4 * HD:6 * HD],
                    in_=wq_r[:, half * 8:(half + 1) * 8, 4 * HD:6 * HD],
                )
            nc.sync.dma_start(
                out=bqk_sb[:], in_=bqk.ap().rearrange("(t p) o -> p (t o)", p=128)
            )
            load_xt(1)
            nc.vector.memset(ones_c[:], 1.0)
            # warmup matmuls: run during the startup DMA wait so the PE
            # HAM clock-gate is already at 8/8 when real work arrives.
            # bd_sb doubles as scratch: memset -> warm reads -> chunk-0 DMA
            # overwrites it (only consumed in section D).
            nc.vector.memset(bd_sb[:, 0:512], 1.0)
            wps = pp.tile([128, 512], f32, tag="work", bufs=2, name="warmps")
            for wi in range(10):
                nc.tensor.matmul(
                    wps[:], lhsT=bd_sb[:, 0:128], rhs=bd_sb[:, 0:512],
                    start=(wi == 0), stop=(wi == 9),
                )

            # ---- phase-1 chunk emission (generator; ~4-MM steps) ----
            def p1_chunk_steps(ci):
                b, nch = divmod(ci, 4)
                qk_sb, v_sb = qk_sbs[b], v_sbs[b]
                ch = slice(nch * 512, (nch + 1) * 512)
                if 1 <= ci and ci + 1 < 2 * (L // 512):
                    load_xt(ci + 1)
                if ci == 0:
                    nc.sync.dma_start(out=cos_sb[:], in_=cosT.ap())
                    nc.sync.dma_start(out=sin_sb[:], in_=sinT.ap())
                    nc.sync.dma_start(out=mask_sb[:], in_=mask0.ap())
                    nc.sync.dma_start(out=bd_sb[:], in_=bdb.ap())
                xt_tiles = xt_store.pop(ci)
                for mp in range(2):
                    ps2 = pp.tile([128, 1024], f32, tag="work", bufs=2,
                                  name=f"qkps{ci}_{mp}")
                    for kt in range(16):
                        xt = xt_tiles[kt // 8]
                        for i in range(2):
                            m = 2 * mp + i
                            nc.tensor.matmul(
                                ps2[:, i * 512:(i + 1) * 512],
                                lhsT=w_sb[:, kt, m * 128:(m + 1) * 128],
                                rhs=xt[:, kt % 8, :],
                                start=(kt == 0),
                                stop=(kt == 15),
                            )
                        if kt % 2 == 1:
                            yield
                    for i in range(2):
                        m = 2 * mp + i
                        nc.scalar.activation(
                            qk_sb[:, m, ch], ps2[:, i * 512:(i + 1) * 512],
                            AFT.Identity, bias=bqk_sb[:, m:m + 1],
                        )
                    yield
                for m in range(4):
                    # fused RoPE on rows 0:R (shuffle via DMA + 3 DVE ops)
                    cs = cos_sb[:, ch]
                    sn = sin_sb[:, ch]
                    rot = p1pool.tile([R, 512], f16, tag="rot", bufs=2)
                    nc.sync.dma_start(out=rot[0:16, :], in_=qk_sb[16:32, m, ch])
                    nc.sync.dma_start(out=rot[16:32, :], in_=qk_sb[0:16, m, ch])
                    nc.vector.tensor_tensor(
                        qk_sb[0:R, m, ch], qk_sb[0:R, m, ch], cs, op=OP.mult
                    )
                    nc.vector.tensor_tensor(rot[:], rot[:], sn, op=OP.mult)
                    nc.vector.tensor_tensor(
                        qk_sb[0:R, m, ch], qk_sb[0:R, m, ch], rot[:], op=OP.add
                    )
                    if m % 2 == 1:
                        yield
                for rr2 in range(2):
                    vpss = [
                        pp.tile([128, 2 * HD], f32,
                                tag=("work" if i else "acc"),
                                bufs=(2 if i else 3),
                                name=f"vps{ci}_{2 * rr2 + i}")
                        for i in range(2)
                    ]
                    for kt in range(16):
                        xt = xt_tiles[kt // 8]
                        for i in range(2):
                            rr = 2 * rr2 + i
                            nc.tensor.matmul(
                                vpss[i][:],
                                lhsT=xt[:, kt % 8, rr * 128:(rr + 1) * 128],
                                rhs=w_sb[:, kt, 4 * HD:6 * HD],
                                start=(kt == 0),
                                stop=(kt == 15),
                            )
                        if kt % 2 == 1:
                            yield
                    for i in range(2):
                        rr = 2 * rr2 + i
                        nc.scalar.activation(
                            v_sb[:, nch * 4 + rr, :], vpss[i][:], AFT.Copy
                        )
                    yield
                # stream Wd behind the x tiles (must land before the A2As:
                # big HBM traffic concurrent with a collective ~doubles it)
                if rr2 == 1:
                    nc.sync.dma_start(
                        out=wd_sb[:, 2 * ci:2 * (ci + 1), :],
                        in_=wd_r[:, 2 * ci:2 * (ci + 1), :],
                    )

            # ---- attention emission (generator; yields per pair) ----
            last_ot = {}

            def attn_steps(b):
                qk_sb, v_sb = qk_sbs[b], v_sbs[b]
                for h in range(HPC):

                    def norm_a(state, b=b, h=h):
                        qc_p, outp_p, sumacc_p = state[:3]
                        sump = pp.tile([1, 512], f32, tag="sump", bufs=1,
                                       name=f"sump{b}_{h}_{qc_p}")
                        nc.tensor.matmul(
                            sump[:], lhsT=ones_c[:], rhs=sumacc_p[:],
                            start=True, stop=True,
                        )
                        rec1 = apool.tile([1, 512], f32, tag="bcs", bufs=1)
                        nc.vector.reciprocal_approx_fast(rec1[:], sump[:])
                        bcs = apool.tile([128, 512], f32, tag="rcp", bufs=1)
                        nc.gpsimd.partition_broadcast(bcs[:], rec1[:])
                        state.append(bcs)

                    def norm_b(state, b=b, h=h):
                        qc_p, outp_p, _, bcs = state
                        ot = apool.tile([128, 512], f16, tag="ot", bufs=2)
                        nc.vector.tensor_tensor(
                            ot[:], outp_p[:], bcs[:], op=OP.mult
                        )
                        for half in range(2):
                            last_ot[(b, h)] = nc.sync.dma_start(
                                out=a2a_ins[b][h][2 * qc_p + half, :, :],
                                in_=ot[:, half * 256:(half + 1) * 256],
                            )

                    prev = None   # [qc, outp, sumacc, (bcs)]
                    for qc in reversed(range(L // 512)):
                        nk = 4 * qc + 4
                        npairs = nk // 2
                        outp = pp.tile([128, 512], f32, tag="acc", bufs=3,
                                       name=f"outp{b}_{h}_{qc}")
                        sumacc = apool.tile([128, 512], f16, tag="sumacc",
                                            bufs=2)

                        def emit_out(pr, outp=outp, nk=nk, v_sb=v_sb, h=h):
                            ka, c0a, npra, kb, c0b, nprb, off, et2 = pr
                            nc.tensor.matmul(
                                outp[:, c0a:512],
                                lhsT=v_sb[:, ka, h * 128:(h + 1) * 128],
                                rhs=et2[:, 0:npra],
                                start=(ka == 0), stop=False,
                            )
                            nc.tensor.matmul(
                                outp[:, c0b:512],
                                lhsT=v_sb[:, kb, h * 128:(h + 1) * 128],
                                rhs=et2[:, off:off + nprb],
                                start=False, stop=(kb == nk - 1),
                            )

                        pend = []
                        for p in range(npairs):
                            ka, kb = 2 * p, 2 * p + 1
                            ja = max(0, ka - qc * 4)
                            jb = max(0, kb - qc * 4)
                            c0a, c0b = ja * 128, jb * 128
                            npra, nprb = 512 - c0a, 512 - c0b
                            off = npra
                            tot = npra + nprb
                            sp2 = pp.tile([128, 1024], f32, tag="work",
                                          bufs=2, name=f"sp{b}_{h}_{qc}_{p}")
                            nc.tensor.matmul(
                                sp2[:, 0:npra],
                                lhsT=qk_sb[:, 2 * h + 1,
                                           ka * 128:(ka + 1) * 128],
                                rhs=qk_sb[:, 2 * h,
                                          qc * 512 + c0a:(qc + 1) * 512],
                                start=True, stop=True,
                            )
                            nc.tensor.matmul(
                                sp2[:, off:off + nprb],
                                lhsT=qk_sb[:, 2 * h + 1,
                                           kb * 128:(kb + 1) * 128],
                                rhs=qk_sb[:, 2 * h,
                                          qc * 512 + c0b:(qc + 1) * 512],
                                start=True, stop=True,
                            )
                            et2 = apool.tile([128, 1024], f16, tag="et",
                                             bufs=3)
                            nc.scalar.activation(
                                et2[:, 0:tot], sp2[:, 0:tot], AFT.Exp,
                                scale=SCALE,
                            )
                            if ka >= qc * 4:
                                nc.vector.tensor_tensor(
                                    et2[:, 0:128], et2[:, 0:128], mask_sb[:],
                                    op=OP.mult,
                                )
                            if kb >= qc * 4:
                                nc.vector.tensor_tensor(
                                    et2[:, off:off + 128],
                                    et2[:, off:off + 128], mask_sb[:],
                                    op=OP.mult,
                                )
                            if p == 0:
                                # first tile always spans all 512 q-cols
                                nc.vector.tensor_copy(
                                    sumacc[:], et2[:, 0:512]
                                )
                            else:
                                nc.vector.tensor_tensor(
                                    sumacc[:, c0a:512], sumacc[:, c0a:512],
                                    et2[:, 0:npra], op=OP.add,
                                )
                            nc.vector.tensor_tensor(
                                sumacc[:, c0b:512], sumacc[:, c0b:512],
                                et2[:, off:off + nprb], op=OP.add,
                            )
                            pend.append(
                                (ka, c0a, npra, kb, c0b, nprb, off, et2)
                            )
                            if len(pend) > 2:
                                emit_out(pend.pop(0))
                            if p == 1 and prev is not None:
                                norm_a(prev)
                            if p == 3 and prev is not None:
                                norm_b(prev)
                                prev = None
                            yield
                        for pr in pend:
                            emit_out(pr)
                            yield
                        if prev is not None:   # qc with only 2 pairs
                            norm_b(prev)
                        prev = [qc, outp, sumacc]
                    norm_a(prev)
                    norm_b(prev)
                    prev = None
                    nc.gpsimd.collective_compute(
                        "AllToAll",
                        mybir.AluOpType.bypass,
                        replica_groups=[CORES],
                        ins=[a2a_ins[b][h][:]],
                        outs=[a2a_outs[b][h][:]],
                    )
                    yield

            # ---- o_sb reshard load groups ----
            o_sbs = [None, None]
            osb_chain = [None]

            def emit_osb_group(b, h, gate, engine):
                if o_sbs[b] is None:
                    o_sbs[b] = qkvpool.tile([128, 16, 256], f16, tag="shr",
                                            bufs=2, name=f"osb{b}")
                for jsrc in range(W):
                    ld = engine.dma_start(
                        out=o_sbs[b][:, 2 * jsrc + h, :],
                        in_=a2a_outs[b][h][jsrc, :, :],
                    )
                    if gate is not None:
                        _add_dep_helper(
                            ld.ins, gate.ins, sync=True,
                            reason="order o_sb after time-critical DMAs",
                        )
                    if engine is nc.sync:
                        if osb_chain[0] is not None:
                            _add_dep_helper(
                                ld.ins, osb_chain[0].ins, sync=True,
                                reason="FIFO-chain o_sb loads",
                            )
                        osb_chain[0] = ld

            # ---- phase-3 emission (generator; yields per half-chain) ----
            def p3_steps(bh):
                for n4 in range(4):
                    for i in range(2):
                        m = 2 * bh + i
                        yp = pp.tile([128, 512], f32, tag="work", bufs=2,
                                     name=f"yps{bh}_{n4}_{m}")
                        for kt in range(16):
                            nc.tensor.matmul(
                                yp[:],
                                lhsT=o_sbs[bh][:, kt, i * 128:(i + 1) * 128],
                                rhs=wd_sb[:, kt, n4 * 512:(n4 + 1) * 512],
                                start=(kt == 0), stop=(kt == 15),
                            )
                            if kt == 7:
                                yield
                        yt = p3s.tile([128, 512], f16, tag="yt", bufs=3)
                        nc.vector.tensor_tensor(
                            yt[:], yp[:], bd_sb[:, n4 * 512:(n4 + 1) * 512],
                            op=OP.add,
                        )
                        nc.sync.dma_start(
                            out=y[m * 128:(m + 1) * 128,
                                  n4 * 512:(n4 + 1) * 512],
                            in_=yt[:],
                        )
                        yield

            def drain(gen):
                for _ in gen:
                    pass

            def pull(gen, n):
                for _ in range(n):
                    if next(gen, StopIteration) is StopIteration:
                        return False
                return True

            # ======== section A: qkv projection b0 ========
            for ci in range(4):
                drain(p1_chunk_steps(ci))

            # ======== section B: attention(b0) x qkv projection(b1) ======
            def p1_rest():
                for ci in range(4, 8):
                    for _ in p1_chunk_steps(ci):
                        yield

            gA = attn_steps(0)
            gP = p1_rest()
            alive = True
            for _ in gA:
                if alive:
                    alive = pull(gP, 2)
            if alive:
                drain(gP)

            # o_sb loads for b0h0 (sync queue, gated after b0h1's ot writes)
            emit_osb_group(0, 0, last_ot[(0, 1)], nc.sync)

            # ======== section C: attention(b1) ========
            gA = attn_steps(1)
            pull(gA, 14)
            emit_osb_group(0, 1, last_ot.get((1, 0)), nc.sync)
            drain(gA)

            # o_sb loads for b1: h0 on sync (gated), h1 on the ACT hw queue
            emit_osb_group(1, 0, last_ot[(1, 1)], nc.sync)
            emit_osb_group(1, 1, None, nc.scalar)

            # ======== section D: out-projection (b0 hides the last A2A) ==
            drain(p3_steps(0))
            # phase3-b1 wave A: 4 chains split even/odd kt. Even kt = head-0
            # channels (ready after the 3rd A2A), so the PE keeps working
            # while the last A2A is still in flight.
            waveA = [(0, 0), (0, 1), (1, 0), (1, 1)]
            ypw = {}
            for idx, (n4, i) in enumerate(waveA):
                yp = pp.tile([128, 512], f32,
                             tag=("work" if idx < 2 else "acc"),
                             bufs=(2 if idx < 2 else 3),
                             name=f"ypw{n4}_{i}")
                for kt in range(0, 16, 2):
                    nc.tensor.matmul(
                        yp[:],
                        lhsT=o_sbs[1][:, kt, i * 128:(i + 1) * 128],
                        rhs=wd_sb[:, kt, n4 * 512:(n4 + 1) * 512],
                        start=(kt == 0), stop=False,
                    )
                ypw[(n4, i)] = yp
            for (n4, i) in waveA:
                yp = ypw[(n4, i)]
                m = 2 + i
                for kt in range(1, 16, 2):
                    nc.tensor.matmul(
                        yp[:],
                        lhsT=o_sbs[1][:, kt, i * 128:(i + 1) * 128],
                        rhs=wd_sb[:, kt, n4 * 512:(n4 + 1) * 512],
                        start=False, stop=(kt == 15),
                    )
                yt = p3s.tile([128, 512], f16, tag="yt", bufs=3)
                nc.vector.tensor_tensor(
                    yt[:], yp[:], bd_sb[:, n4 * 512:(n4 + 1) * 512],
                    op=OP.add,
                )
                nc.sync.dma_start(
                    out=y[m * 128:(m + 1) * 128, n4 * 512:(n4 + 1) * 512],
                    in_=yt[:],
                )
            # wave B: remaining 4 chains, contiguous
            for n4 in (2, 3):
                for i in range(2):
                    m = 2 + i
                    yp = pp.tile([128, 512], f32, tag="work", bufs=2,
                                 name=f"ypb{n4}_{i}")
                    for kt in range(16):
                        nc.tensor.matmul(
                            yp[:],
                            lhsT=o_sbs[1][:, kt, i * 128:(i + 1) * 128],
                            rhs=wd_sb[:, kt, n4 * 512:(n4 + 1) * 512],
                            start=(kt == 0), stop=(kt == 15),
                        )
                    yt = p3s.tile([128, 512], f16, tag="yt", bufs=3)
                    nc.vector.tensor_tensor(
                        yt[:], yp[:], bd_sb[:, n4 * 512:(n4 + 1) * 512],
                        op=OP.add,
                    )
                    nc.sync.dma_start(
                        out=y[m * 128:(m + 1) * 128,
                              n4 * 512:(n4 + 1) * 512],
                        in_=yt[:],
                    )
    nc.finalize()
    return nc


def _host_prep(x_BLD, cos, sin, Wqkv, bqkv, Wd, bd):
    x = np.asarray(x_BLD, np.float32).reshape(M, D)
    xT = np.ascontiguousarray(x.T.astype(np.float16))
    cosT = np.ascontiguousarray(
        np.asarray(cos, np.float32).reshape(L, R).T.astype(np.float16)
    )
    s2 = np.asarray(sin, np.float32).reshape(L, R).T
    sinT_pm = np.ascontiguousarray(
        np.concatenate([-s2[:16], s2[16:]], axis=0).astype(np.float16)
    )
    kk = np.arange(128, dtype=np.int64)[:, None]
    qq = np.arange(128, dtype=np.int64)[None, :]
    mask0 = np.ascontiguousarray((qq >= kk).astype(np.float16))
    Wqkv = np.asarray(Wqkv, np.float32)
    bqkv = np.asarray(bqkv, np.float32)
    # softmax rows sum to 1, so the v-bias contribution collapses to a
    # constant vector bv_glob @ Wd folded into bd
    HDl = D // H
    v_bias = np.concatenate(
        [bqkv[hh * 3 * HDl + 2 * HDl:(hh * 3 + 3) * HDl] for hh in range(H)]
    )
    bd_eff = np.asarray(bd, np.float32) + v_bias @ np.asarray(Wd, np.float32)
    bdb = np.ascontiguousarray(
        np.broadcast_to(bd_eff.astype(np.float16), (128, D))
    )
    in_maps = []
    for c in range(W):
        base = c * HPC * 3 * HD
        qk_idx = np.concatenate(
            [np.arange(base + h * 3 * HD, base + h * 3 * HD + 2 * HD)
             for h in range(HPC)]
        )
        v_idx = np.concatenate(
            [np.arange(base + h * 3 * HD + 2 * HD, base + (h + 1) * 3 * HD)
             for h in range(HPC)]
        )
        in_maps.append({
            "xT": xT,
            "wqkv": np.ascontiguousarray(
                Wqkv[:, np.concatenate([qk_idx, v_idx])].astype(np.float16)
            ),
            "bqk": np.ascontiguousarray(bqkv[qk_idx].reshape(4 * HD, 1)),
            "cosT": cosT,
            "sinT": sinT_pm,
            "mask0": mask0,
            "wd": np.asarray(Wd, np.float32).astype(np.float16),
            "bdb": bdb,
        })
    return in_maps


def _get_nc():
    global _NC
    if _NC is None:
        _NC = _build_nc()
    return _NC


def _run(inputs, trace=False, tmpdir=None):
    from concourse.bass_utils import run_bass_kernel_spmd

    in_maps = _host_prep(**inputs)
    nc = _get_nc()
    res = run_bass_kernel_spmd(nc, in_maps, CORES, trace=trace, tmpdir=tmpdir)
    out = np.empty((M, D), np.float32)
    for c in CORES:
        yc = np.asarray(res.results[c]["y"], np.float32)  # [512, D]
        out[c * 256:(c + 1) * 256] = yc[:256]
        out[L + c * 256:L + (c + 1) * 256] = yc[256:]
    return out.reshape(B, L, D), res


def kernel(**inputs) -> np.ndarray:
    out, _ = _run(inputs)
    return out



# revision 18
# speedup vs baseline: 1.1383x; 1.0017x over previous
"""Distributed Trainium2 kernel for nn_DecoderAttentionRotary.

Strategy (8 NeuronCores, tensor-parallel over heads, fp16 matmul datapath):
  - host: transpose x -> xT [D, B*L] fp16; per-core Wqkv column slice
    reordered to [q0,k0,q1,k1,v0|v1] fp16; cos/sin transposed fp16; one
    128x128 causal mask; v-bias folded into bd (softmax rows sum to 1,
    so attn@(xWv+1*bv)@Wd = attn@xWv@Wd + bv@Wd).
  - device, per core (2 heads), with CROSS-PHASE INTERLEAVED emission
    (attention is ACT/exp-throughput-bound, projections are PE-bound):
      section A: qkv projection b0 (startup DMAs finely interleaved,
                 v-columns of W deferred),
      section B: attention(b0) interleaved ~1:2 with qkv projection(b1);
                 the b0 AllToAlls are fully hidden here,
      section C: attention(b1) compressed (fast A2As),
      section D: out-projection; b0's half hides the last A2A, and the
                 first 4 chains of b1's half are split even/odd k so the
                 even (head-0) halves also run before the last A2A lands.
  - attention: scores^T layout; k-tiles in PAIRS sharing one [128,1024]
    PSUM tile and ONE packed exp (amortizes the ~260ns ACT instruction
    overhead); out-matmuls lag 2 pairs behind scores; row-sums
    accumulated on DVE (sumacc += et), reduced by a single ones-matmul
    per q-chunk + fast approximate reciprocal + gpsimd broadcast, all
    staggered one q-chunk behind compute so nothing waits on the chain.
  - per-(batch,head) AllToAll reshard (fp16); o_sb reshard loads are
    FIFO-chained plain copies explicitly dep-ordered after the next
    head's attention-output DMAs (the tile scheduler otherwise hoists
    them where their semaphore waits head-of-line-block the sync queue);
    the final group is issued from the ACT hardware DMA queue.
  - y computed in fp16 (halves output DMA), upcast to fp32 on host.
  - host: scatter the per-core 256-row halves into the full output.
"""
import sys

for _p in ("/opt/pypackages", "/opt/trn_rl_repo"):
    if _p not in sys.path:
        sys.path.insert(0, _p)

import numpy as np

B, L, D, H = 2, 2048, 2048, 16
HD, R = 128, 32
SCALE = float(HD) ** -0.5
W = 8
HPC = H // W              # heads per core
M = B * L                 # flattened rows
CORES = list(range(W))

_NC = None


def _build_nc():
    import concourse.mybir as mybir
    import concourse.tile as tile
    from concourse import bacc
    from concourse.bass import _add_dep_helper
    from concourse.bass_isa import ReduceOp

    f32 = mybir.dt.float32
    f16 = mybir.dt.float16
    AFT = mybir.ActivationFunctionType
    OP = mybir.AluOpType

    nc = bacc.Bacc(None, target_bir_lowering=False, num_devices=W)
    # host pre-tiles x / weights so every DMA is one contiguous DRAM read
    xP = nc.declare_dram_parameter("xP", [8, 4, 128, 4, 512], f16,
                                   isOutput=False)
    wqk = nc.declare_dram_parameter("wqk", [128, 16, 4 * HD], f16,
                                    isOutput=False)
    wv = nc.declare_dram_parameter("wv", [128, 16, 2 * HD], f16,
                                   isOutput=False)
    bqk = nc.declare_dram_parameter("bqk", [4 * HD, 1], f32, isOutput=False)
    cosT = nc.declare_dram_parameter("cosT", [R, L], f16, isOutput=False)
    sinT = nc.declare_dram_parameter("sinT", [R, L], f16, isOutput=False)
    mask0 = nc.declare_dram_parameter("mask0", [128, 128], f16, isOutput=False)
    wdP = nc.declare_dram_parameter("wdP", [128, 16, D], f16, isOutput=False)
    bdb = nc.declare_dram_parameter("bdb", [128, D], f16, isOutput=False)
    y = nc.declare_dram_parameter("y", [M // W, D], f16, isOutput=True)

    with tile.TileContext(nc) as tc:
        with (
            tc.tile_pool(name="const", bufs=1) as cpool,
            tc.tile_pool(name="dram", bufs=1, space="DRAM") as dpool,
            tc.tile_pool(name="ps", bufs=1, space="PSUM") as pp,
            tc.tile_pool(name="qkv", bufs=1) as qkvpool,
            tc.tile_pool(name="p3s", bufs=2) as p3s,
            tc.tile_pool(name="att", bufs=2) as apool,
            tc.tile_pool(name="p1", bufs=2) as p1pool,
        ):
            a2a_ins = [[dpool.tile([W, HD, 256], f16, name=f"a2ain{b}_{h}")
                        for h in range(HPC)] for b in range(B)]
            a2a_outs = [[dpool.tile([W, HD, 256], f16, name=f"a2aout{b}_{h}")
                         for h in range(HPC)] for b in range(B)]

            w_sb = cpool.tile([128, 16, 6 * HD], f16)
            wd_sb = cpool.tile([128, 16, D], f16)
            bd_sb = cpool.tile([128, D], f16)
            bqk_sb = cpool.tile([128, 4], f32)
            ones_c = cpool.tile([128, 1], f16)
            cos_sb = qkvpool.tile([R, L], f16, tag="shr", bufs=2,
                                  padded_shape=[128, 16 * 256])
            sin_sb = qkvpool.tile([R, L], f16, tag="shr", bufs=2,
                                  padded_shape=[128, 16 * 256])
            mask_sb = cpool.tile([128, 128], f16)
            qk_sbs, v_sbs = [], []
            for b in range(B):
                qk_sbs.append(qkvpool.tile([128, 4, L], f16, name=f"qk{b}"))
                v_sbs.append(qkvpool.tile([128, 16, 2 * HD], f16, name=f"v{b}"))

            # ---- startup ----
            # sync queue carries ONLY xt pieces (big, latency-critical);
            # weights/consts go on the scalar queue so neither FIFO
            # head-blocks the other.  xt double-buffer depth is 2 full
            # chunks (bufs=4) so a chunk's loads never WAR-wait on the
            # previous chunk's reads and always land early.
            xt_store = {}

            def load_xt(ci):
                b, nch = divmod(ci, 4)
                n0 = b * L + nch * 512
                tiles = []
                for half in range(2):
                    xt = p1pool.tile([128, 8, 512], f16, tag="xt", bufs=4,
                                     name=f"xt{ci}_{half}")
                    for piece in range(2):
                        nc.sync.dma_start(
                            out=xt[:, piece * 4:(piece + 1) * 4, :],
                            in_=xP.ap()[ci, half * 2 + piece],
                        )
                    tiles.append(xt)
                xt_store[ci] = tiles

            xt0_tiles = []
            for half in range(2):
                xt = p1pool.tile([128, 8, 512], f16, tag="xt", bufs=4,
                                 name=f"xt0_{half}")
                xt0_tiles.append(xt)
            # startup pieces interleaved on sync, consumption-ordered;
            # first pieces tiny so the first matmul can start ASAP
            def xt0_piece(pi, t0=None, t1=None):
                src = xP.ap()[0, pi]
                o0, o1 = pi * 4, (pi + 1) * 4
                if t0 is not None:
                    src = src[:, t0:t1, :]
                    o0, o1 = pi * 4 + t0, pi * 4 + t1
                nc.sync.dma_start(
                    out=xt0_tiles[pi // 2][:, o0 - 8 * (pi // 2):
                                           o1 - 8 * (pi // 2), :],
                    in_=src,
                )

            def w_piece(k0, k1):
                nc.sync.dma_start(
                    out=w_sb[:, k0:k1, 0:4 * HD],
                    in_=wqk.ap()[:, k0:k1, :],
                )

            xt0_piece(0, 0, 1)
            w_piece(0, 1)
            w_piece(1, 4)
            xt0_piece(0, 1, 4)
            w_piece(4, 8)
            xt0_piece(1)
            w_piece(8, 12)
            xt0_piece(2)
            w_piece(12, 16)
            xt0_piece(3)
            xt_store[0] = xt0_tiles
            for half in range(2):
                nc.sync.dma_start(
                    out=w_sb[:, half * 8:(half + 1) * 8, 4 * HD:6 * HD],
                    in_=wv.ap()[:, half * 8:(half + 1) * 8, :],
                )
            nc.sync.dma_start(
                out=bqk_sb[:], in_=bqk.ap().rearrange("(t p) o -> p (t o)", p=128)
            )
            load_xt(1)
            nc.vector.memset(ones_c[:], 1.0)
            # warmup matmuls: run during the startup DMA wait so the PE
            # HAM clock-gate is already at 8/8 when real work arrives.
            # bd_sb doubles as scratch: memset -> warm reads -> chunk-0 DMA
            # overwrites it (only consumed in section D).
            nc.vector.memset(bd_sb[:, 0:512], 1.0)
            wps = pp.tile([128, 512], f32, tag="work", bufs=2, name="warmps")
            for wi in range(10):
                nc.tensor.matmul(
                    wps[:], lhsT=bd_sb[:, 0:128], rhs=bd_sb[:, 0:512],
                    start=(wi == 0), stop=(wi == 9),
                )

            # ---- phase-1 chunk emission (generator; ~4-MM steps) ----
            def p1_chunk_steps(ci):
                b, nch = divmod(ci, 4)
                qk_sb, v_sb = qk_sbs[b], v_sbs[b]
                ch = slice(nch * 512, (nch + 1) * 512)
                if 1 <= ci and ci + 1 < 2 * (L // 512):
                    load_xt(ci + 1)
                if ci == 0:
                    nc.sync.dma_start(out=cos_sb[:], in_=cosT.ap())
                    nc.sync.dma_start(out=sin_sb[:], in_=sinT.ap())
                    nc.sync.dma_start(out=mask_sb[:], in_=mask0.ap())
                    nc.sync.dma_start(out=bd_sb[:], in_=bdb.ap())
                xt_tiles = xt_store.pop(ci)
                for mp in range(2):
                    ps2 = pp.tile([128, 1024], f32, tag="work", bufs=2,
                                  name=f"qkps{ci}_{mp}")
                    for kt in range(16):
                        xt = xt_tiles[kt // 8]
                        for i in range(2):
                            m = 2 * mp + i
                            nc.tensor.matmul(
                                ps2[:, i * 512:(i + 1) * 512],
                                lhsT=w_sb[:, kt, m * 128:(m + 1) * 128],
                                rhs=xt[:, kt % 8, :],
                                start=(kt == 0),
                                stop=(kt == 15),
                            )
                        if kt % 2 == 1:
                            yield
                    for i in range(2):
                        m = 2 * mp + i
                        nc.scalar.activation(
                            qk_sb[:, m, ch], ps2[:, i * 512:(i + 1) * 512],
                            AFT.Identity, bias=bqk_sb[:, m:m + 1],
                        )
                    yield
                for m in range(4):
                    # fused RoPE on rows 0:R (shuffle via DMA + 3 DVE ops)
                    cs = cos_sb[:, ch]
                    sn = sin_sb[:, ch]
                    rot = p1pool.tile([R, 512], f16, tag="rot", bufs=2)
                    nc.sync.dma_start(out=rot[0:16, :], in_=qk_sb[16:32, m, ch])
                    nc.sync.dma_start(out=rot[16:32, :], in_=qk_sb[0:16, m, ch])
                    nc.vector.tensor_tensor(
                        qk_sb[0:R, m, ch], qk_sb[0:R, m, ch], cs, op=OP.mult
                    )
                    nc.vector.tensor_tensor(rot[:], rot[:], sn, op=OP.mult)
                    nc.vector.tensor_tensor(
                        qk_sb[0:R, m, ch], qk_sb[0:R, m, ch], rot[:], op=OP.add
                    )
                    if m % 2 == 1:
                        yield
                for rr2 in range(2):
                    vpss = [
                        pp.tile([128, 2 * HD], f32,
                                tag=("work" if i else "acc"),
                                bufs=(2 if i else 3),
                                name=f"vps{ci}_{2 * rr2 + i}")
                        for i in range(2)
                    ]
                    for kt in range(16):
                        xt = xt_tiles[kt // 8]
                        for i in range(2):
                            rr = 2 * rr2 + i
                            nc.tensor.matmul(
                                vpss[i][:],
                                lhsT=xt[:, kt % 8, rr * 128:(rr + 1) * 128],
                                rhs=w_sb[:, kt, 4 * HD:6 * HD],
                                start=(kt == 0),
                                stop=(kt == 15),
                            )
                        if kt % 2 == 1:
                            yield
                    for i in range(2):
                        rr = 2 * rr2 + i
                        nc.scalar.activation(
                            v_sb[:, nch * 4 + rr, :], vpss[i][:], AFT.Copy
                        )
                    yield
                # stream Wd behind the x tiles (must land before the A2As:
                # big HBM traffic concurrent with a collective ~doubles it)
                if rr2 == 1:
                    nc.sync.dma_start(
                        out=wd_sb[:, 2 * ci:2 * (ci + 1), :],
                        in_=wdP.ap()[:, 2 * ci:2 * (ci + 1), :],
                    )

            # ---- attention emission (generator; yields per pair) ----
            last_ot = {}

            def attn_steps(b):
                qk_sb, v_sb = qk_sbs[b], v_sbs[b]
                for h in range(HPC):

                    def norm_a(state, b=b, h=h):
                        qc_p, outp_p, sumacc_p = state[:3]
                        sump = pp.tile([1, 512], f32, tag="sump", bufs=1,
                                       name=f"sump{b}_{h}_{qc_p}")
                        nc.tensor.matmul(
                            sump[:], lhsT=ones_c[:], rhs=sumacc_p[:],
                            start=True, stop=True,
                        )
                        rec1 = apool.tile([1, 512], f32, tag="bcs", bufs=1)
                        nc.vector.reciprocal_approx_fast(rec1[:], sump[:])
                        bcs = apool.tile([128, 512], f32, tag="rcp", bufs=1)
                        nc.gpsimd.partition_broadcast(bcs[:], rec1[:])
                        state.append(bcs)

                    def norm_b(state, b=b, h=h):
                        qc_p, outp_p, _, bcs = state
                        ot = apool.tile([128, 512], f16, tag="ot", bufs=2)
                        nc.vector.tensor_tensor(
                            ot[:], outp_p[:], bcs[:], op=OP.mult
                        )
                        for half in range(2):
                            last_ot[(b, h)] = nc.sync.dma_start(
                                out=a2a_ins[b][h][2 * qc_p + half, :, :],
                                in_=ot[:, half * 256:(half + 1) * 256],
                            )

                    prev = None   # [qc, outp, sumacc, (bcs)]
                    for qc in reversed(range(L // 512)):
                        nk = 4 * qc + 4
                        npairs = nk // 2
                        outp = pp.tile([128, 512], f32, tag="acc", bufs=3,
                                       name=f"outp{b}_{h}_{qc}")
                        sumacc = apool.tile([128, 512], f16, tag="sumacc",
                                            bufs=2)

                        def emit_out(pr, outp=outp, nk=nk, v_sb=v_sb, h=h):
                            ka, c0a, npra, kb, c0b, nprb, off, et2 = pr
                            nc.tensor.matmul(
                                outp[:, c0a:512],
                                lhsT=v_sb[:, ka, h * 128:(h + 1) * 128],
                                rhs=et2[:, 0:npra],
                                start=(ka == 0), stop=False,
                            )
                            nc.tensor.matmul(
                                outp[:, c0b:512],
                                lhsT=v_sb[:, kb, h * 128:(h + 1) * 128],
                                rhs=et2[:, off:off + nprb],
                                start=False, stop=(kb == nk - 1),
                            )

                        pend = []
                        for p in range(npairs):
                            ka, kb = 2 * p, 2 * p + 1
                            ja = max(0, ka - qc * 4)
                            jb = max(0, kb - qc * 4)
                            c0a, c0b = ja * 128, jb * 128
                            npra, nprb = 512 - c0a, 512 - c0b
                            off = npra
                            tot = npra + nprb
                            sp2 = pp.tile([128, 1024], f32, tag="work",
                                          bufs=2, name=f"sp{b}_{h}_{qc}_{p}")
                            nc.tensor.matmul(
                                sp2[:, 0:npra],
                                lhsT=qk_sb[:, 2 * h + 1,
                                           ka * 128:(ka + 1) * 128],
                                rhs=qk_sb[:, 2 * h,
                                          qc * 512 + c0a:(qc + 1) * 512],
                                start=True, stop=True,
                            )
                            nc.tensor.matmul(
                                sp2[:, off:off + nprb],
                                lhsT=qk_sb[:, 2 * h + 1,
                                           kb * 128:(kb + 1) * 128],
                                rhs=qk_sb[:, 2 * h,
                                          qc * 512 + c0b:(qc + 1) * 512],
                                start=True, stop=True,
                            )
                            et2 = apool.tile([128, 1024], f16, tag="et",
                                             bufs=3)
                            nc.scalar.activation(
                                et2[:, 0:tot], sp2[:, 0:tot], AFT.Exp,
                                scale=SCALE,
                            )
                            if ka >= qc * 4:
                                nc.vector.tensor_tensor(
                                    et2[:, 0:128], et2[:, 0:128], mask_sb[:],
                                    op=OP.mult,
                                )
                            if kb >= qc * 4:
                                nc.vector.tensor_tensor(
                                    et2[:, off:off + 128],
                                    et2[:, off:off + 128], mask_sb[:],
                                    op=OP.mult,
                                )
                            if p == 0:
                                # first tile always spans all 512 q-cols
                                nc.vector.tensor_copy(
                                    sumacc[:], et2[:, 0:512]
                                )
                            else:
                                nc.vector.tensor_tensor(
                                    sumacc[:, c0a:512], sumacc[:, c0a:512],
                                    et2[:, 0:npra], op=OP.add,
                                )
                            nc.vector.tensor_tensor(
                                sumacc[:, c0b:512], sumacc[:, c0b:512],
                                et2[:, off:off + nprb], op=OP.add,
                            )
                            pend.append(
                                (ka, c0a, npra, kb, c0b, nprb, off, et2)
                            )
                            if len(pend) > 2:
                                emit_out(pend.pop(0))
                            if p == 1 and prev is not None:
                                norm_a(prev)
                            if p == 3 and prev is not None:
                                norm_b(prev)
                                prev = None
                            yield
                        for pr in pend:
                            emit_out(pr)
                            yield
                        if prev is not None:   # qc with only 2 pairs
                            norm_b(prev)
                        prev = [qc, outp, sumacc]
                    norm_a(prev)
                    norm_b(prev)
                    prev = None
                    nc.gpsimd.collective_compute(
                        "AllToAll",
                        mybir.AluOpType.bypass,
                        replica_groups=[CORES],
                        ins=[a2a_ins[b][h][:]],
                        outs=[a2a_outs[b][h][:]],
                    )
                    yield

            # ---- o_sb reshard load groups ----
            o_sbs = [None, None]
            osb_chain = [None]

            def emit_osb_group(b, h, gate, engine):
                if o_sbs[b] is None:
                    o_sbs[b] = qkvpool.tile([128, 16, 256], f16, tag="shr",
                                            bufs=2, name=f"osb{b}")
                for jsrc in range(W):
                    ld = engine.dma_start(
                        out=o_sbs[b][:, 2 * jsrc + h, :],
                        in_=a2a_outs[b][h][jsrc, :, :],
                    )
                    if gate is not None:
                        _add_dep_helper(
                            ld.ins, gate.ins, sync=True,
                            reason="order o_sb after time-critical DMAs",
                        )
                    if engine is nc.sync:
                        if osb_chain[0] is not None:
                            _add_dep_helper(
                                ld.ins, osb_chain[0].ins, sync=True,
                                reason="FIFO-chain o_sb loads",
                            )
                        osb_chain[0] = ld

            # ---- phase-3 emission (generator; yields per half-chain) ----
            def p3_steps(bh):
                for n4 in range(4):
                    for i in range(2):
                        m = 2 * bh + i
                        yp = pp.tile([128, 512], f32, tag="work", bufs=2,
                                     name=f"yps{bh}_{n4}_{m}")
                        for kt in range(16):
                            nc.tensor.matmul(
                                yp[:],
                                lhsT=o_sbs[bh][:, kt, i * 128:(i + 1) * 128],
                                rhs=wd_sb[:, kt, n4 * 512:(n4 + 1) * 512],
                                start=(kt == 0), stop=(kt == 15),
                            )
                            if kt == 7:
                                yield
                        yt = p3s.tile([128, 512], f16, tag="yt", bufs=3)
                        nc.vector.tensor_tensor(
                            yt[:], yp[:], bd_sb[:, n4 * 512:(n4 + 1) * 512],
                            op=OP.add,
                        )
                        nc.sync.dma_start(
                            out=y[m * 128:(m + 1) * 128,
                                  n4 * 512:(n4 + 1) * 512],
                            in_=yt[:],
                        )
                        yield

            def drain(gen):
                for _ in gen:
                    pass

            def pull(gen, n):
                for _ in range(n):
                    if next(gen, StopIteration) is StopIteration:
                        return False
                return True

            # ======== section A: qkv projection b0 ========
            for ci in range(4):
                drain(p1_chunk_steps(ci))

            # ======== section B: attention(b0) x qkv projection(b1) ======
            def p1_rest():
                for ci in range(4, 8):
                    for _ in p1_chunk_steps(ci):
                        yield

            gA = attn_steps(0)
            gP = p1_rest()
            alive = True
            for _ in gA:
                if alive:
                    alive = pull(gP, 2)
            if alive:
                drain(gP)

            # o_sb loads for b0h0 (sync queue, gated after b0h1's ot writes)
            emit_osb_group(0, 0, last_ot[(0, 1)], nc.sync)

            # ======== section C: attention(b1) ========
            gA = attn_steps(1)
            pull(gA, 14)
            emit_osb_group(0, 1, last_ot.get((1, 0)), nc.sync)
            drain(gA)

            # o_sb loads for b1: h0 on sync (gated), h1 on the ACT hw queue
            emit_osb_group(1, 0, last_ot[(1, 1)], nc.sync)
            emit_osb_group(1, 1, None, nc.scalar)

            # ======== section D: out-projection (b0 hides the last A2A) ==
            drain(p3_steps(0))
            # phase3-b1 wave A: 4 chains split even/odd kt. Even kt = head-0
            # channels (ready after the 3rd A2A), so the PE keeps working
            # while the last A2A is still in flight.
            waveA = [(0, 0), (0, 1), (1, 0), (1, 1)]
            ypw = {}
            for idx, (n4, i) in enumerate(waveA):
                yp = pp.tile([128, 512], f32,
                             tag=("work" if idx < 2 else "acc"),
                             bufs=(2 if idx < 2 else 3),
                             name=f"ypw{n4}_{i}")
                for kt in range(0, 16, 2):
                    nc.tensor.matmul(
                        yp[:],
                        lhsT=o_sbs[1][:, kt, i * 128:(i + 1) * 128],
                        rhs=wd_sb[:, kt, n4 * 512:(n4 + 1) * 512],
                        start=(kt == 0), stop=False,
                    )
                ypw[(n4, i)] = yp
            for (n4, i) in waveA:
                yp = ypw[(n4, i)]
                m = 2 + i
                for kt in range(1, 16, 2):
                    nc.tensor.matmul(
                        yp[:],
                        lhsT=o_sbs[1][:, kt, i * 128:(i + 1) * 128],
                        rhs=wd_sb[:, kt, n4 * 512:(n4 + 1) * 512],
                        start=False, stop=(kt == 15),
                    )
                yt = p3s.tile([128, 512], f16, tag="yt", bufs=3)
                nc.vector.tensor_tensor(
                    yt[:], yp[:], bd_sb[:, n4 * 512:(n4 + 1) * 512],
                    op=OP.add,
                )
                nc.sync.dma_start(
                    out=y[m * 128:(m + 1) * 128, n4 * 512:(n4 + 1) * 512],
                    in_=yt[:],
                )
            # wave B: remaining 4 chains, contiguous
            for n4 in (2, 3):
                for i in range(2):
                    m = 2 + i
                    yp = pp.tile([128, 512], f32, tag="work", bufs=2,
                                 name=f"ypb{n4}_{i}")
                    for kt in range(16):
                        nc.tensor.matmul(
                            yp[:],
                            lhsT=o_sbs[1][:, kt, i * 128:(i + 1) * 128],
                            rhs=wd_sb[:, kt, n4 * 512:(n4 + 1) * 512],
                            start=(kt == 0), stop=(kt == 15),
                        )
                    yt = p3s.tile([128, 512], f16, tag="yt", bufs=3)
                    nc.vector.tensor_tensor(
                        yt[:], yp[:], bd_sb[:, n4 * 512:(n4 + 1) * 512],
                        op=OP.add,
                    )
                    nc.sync.dma_start(
                        out=y[m * 128:(m + 1) * 128,
                              n4 * 512:(n4 + 1) * 512],
                        in_=yt[:],
                    )
    nc.finalize()
    return nc


def _host_prep(x_BLD, cos, sin, Wqkv, bqkv, Wd, bd):
    x = np.asarray(x_BLD, np.float32).reshape(M, D)
    xT = np.ascontiguousarray(x.T.astype(np.float16))
    cosT = np.ascontiguousarray(
        np.asarray(cos, np.float32).reshape(L, R).T.astype(np.float16)
    )
    s2 = np.asarray(sin, np.float32).reshape(L, R).T
    sinT_pm = np.ascontiguousarray(
        np.concatenate([-s2[:16], s2[16:]], axis=0).astype(np.float16)
    )
    kk = np.arange(128, dtype=np.int64)[:, None]
    qq = np.arange(128, dtype=np.int64)[None, :]
    mask0 = np.ascontiguousarray((qq >= kk).astype(np.float16))
    Wqkv = np.asarray(Wqkv, np.float32)
    bqkv = np.asarray(bqkv, np.float32)
    # softmax rows sum to 1, so the v-bias contribution collapses to a
    # constant vector bv_glob @ Wd folded into bd
    HDl = D // H
    v_bias = np.concatenate(
        [bqkv[hh * 3 * HDl + 2 * HDl:(hh * 3 + 3) * HDl] for hh in range(H)]
    )
    bd_eff = np.asarray(bd, np.float32) + v_bias @ np.asarray(Wd, np.float32)
    bdb = np.ascontiguousarray(
        np.broadcast_to(bd_eff.astype(np.float16), (128, D))
    )
    # contiguous-DMA tilings: xP[c, piece, p, t, n] = x[c*512+n, (4pc+t)*128+p]
    xP = np.ascontiguousarray(
        xT.reshape(4, 4, 128, 8, 512).transpose(3, 0, 2, 1, 4)
    )
    wdH = np.ascontiguousarray(
        np.asarray(Wd, np.float32).astype(np.float16)
        .reshape(16, 128, D).transpose(1, 0, 2)
    )
    in_maps = []
    for c in range(W):
        base = c * HPC * 3 * HD
        qk_idx = np.concatenate(
            [np.arange(base + h * 3 * HD, base + h * 3 * HD + 2 * HD)
             for h in range(HPC)]
        )
        v_idx = np.concatenate(
            [np.arange(base + h * 3 * HD + 2 * HD, base + (h + 1) * 3 * HD)
             for h in range(HPC)]
        )
        wqkH = np.ascontiguousarray(
            Wqkv[:, qk_idx].astype(np.float16)
            .reshape(16, 128, 4 * HD).transpose(1, 0, 2)
        )
        wvH = np.ascontiguousarray(
            Wqkv[:, v_idx].astype(np.float16)
            .reshape(16, 128, 2 * HD).transpose(1, 0, 2)
        )
        in_maps.append({
            "xP": xP,
            "wqk": wqkH,
            "wv": wvH,
            "bqk": np.ascontiguousarray(bqkv[qk_idx].reshape(4 * HD, 1)),
            "cosT": cosT,
            "sinT": sinT_pm,
            "mask0": mask0,
            "wdP": wdH,
            "bdb": bdb,
        })
    return in_maps


def _get_nc():
    global _NC
    if _NC is None:
        _NC = _build_nc()
    return _NC


def _run(inputs, trace=False, tmpdir=None):
    from concourse.bass_utils import run_bass_kernel_spmd

    in_maps = _host_prep(**inputs)
    nc = _get_nc()
    res = run_bass_kernel_spmd(nc, in_maps, CORES, trace=trace, tmpdir=tmpdir)
    out = np.empty((M, D), np.float32)
    for c in CORES:
        yc = np.asarray(res.results[c]["y"], np.float32)  # [512, D]
        out[c * 256:(c + 1) * 256] = yc[:256]
        out[L + c * 256:L + (c + 1) * 256] = yc[256:]
    return out.reshape(B, L, D), res


def kernel(**inputs) -> np.ndarray:
    out, _ = _run(inputs)
    return out

